# revision 34
# baseline (speedup 1.0000x reference)
"""Trainium2 Bass kernel for nn_AppearanceLoss (keypoint patch CNN MSE).

Host: crops 33x33 patches at keypoint locations, builds full im2col
(27 rows = 3c x 3dy x 3dx per patch) so conv1 is a single-shot matmul,
shards 256 keypoints across 8 NeuronCores.

Device (v2): group-PAIR structure (2 groups = 16 patches per iteration,
ground/satellite pairs interleaved so the MSE endgame chunks through the
kernel). conv1 = 64x64 PE-tiling, 4 concurrent tiles; conv2 =
offset-accumulated K=64 block-diag matmuls, one weight load per
(offset, row-half) serving both groups of the pair (a post-build pass
deletes redundant LDWEIGHTS). conv2 eviction = relu+bias to bf16
scratch on ACT/DVE + segmented GAP reduce on GpSimd. Linear on feature
diffs computed in 4 chunks during steady state; output DMA split in two.
Host sums 8 per-core partials into the scalar MSE.
"""

import sys

sys.path.insert(0, "/opt/trn_rl_repo")

from contextlib import ExitStack

import ml_dtypes
import numpy as np

import concourse.bass as bass  # noqa: F401
import concourse.tile as tile
from concourse import bacc, bass_utils, mybir

SIGMA = 16
PATCH = 33  # 2*SIGMA+1
HOUT = 31  # conv1 valid output: 33-3+1
COUT = 15  # conv2 stride-2 valid output: (31-3)//2+1
B, K, H = 4, 64, 256
NCORES = 8
NKP = B * K  # 256 keypoints total
KPC = NKP // NCORES  # 32 keypoints per core
NPATCH = KPC * B  # 128 patches per core per set
NG = 32  # groups of 8 patches (16 ground + 16 sat)
NGP = NG // 2  # 16 group-pairs
KIM = 55  # conv1 im2col rows per pair: 2*27 + ones
BF16 = mybir.dt.bfloat16
F32 = mybir.dt.float32
NPBF16 = ml_dtypes.bfloat16

_CACHE: dict = {}


def _dedupe_ldweights(nc):
    """Remove InstLdweights that reload weights already resident in the
    same PE tile position (identical access pattern, no intervening
    overlapping load). Waits on a removed load move to the next
    instruction (its matmul); loads with updates are kept."""
    removed = 0
    for blk in nc.main_func.blocks:
        referenced = set()
        for inst in blk.instructions:
            try:
                for name, _ in inst.dependency_edges():
                    referenced.add(name)
            except Exception:
                pass
        live = {}  # tile_position -> (signature, rect)

        def overlap(a, b):
            return a[0] < b[1] and b[0] < a[1] and a[2] < b[3] and b[2] < a[3]

        insts = blk.instructions
        keep = []
        for idx, inst in enumerate(insts):
            tname = type(inst).__name__
            if tname != "InstLdweights":
                keep.append(inst)
                continue
            tp = inst.tile_position
            ts = inst.tile_size
            if tp is None or ts is None:
                live.clear()
                keep.append(inst)
                continue
            rect = (tp[0], tp[0] + ts[0], tp[1], tp[1] + ts[1])
            sig = (
                tuple(tp),
                tuple(ts),
                inst.perf_mode,
                inst.is_transpose,
                str(inst.ins[0]),
            )
            si = inst.sync_info
            has_update = si is not None and len(si.on_update) > 0
            prev = live.get(tuple(tp))
            if (
                prev is not None
                and prev[0] == sig
                and not has_update
                and inst.name not in referenced
            ):
                # redundant: same weights already loaded at this position
                waits = list(si.on_wait) if si is not None else []
                if waits:
                    # move waits onto the next instruction (its matmul)
                    nxt = insts[idx + 1]
                    nsi = nxt.sync_info
                    if nsi is None:
                        nxt.sync_info = mybir.SyncInfo(
                            on_wait=waits, on_update=[]
                        )
                    else:
                        nsi.on_wait = list(nsi.on_wait) + waits
                removed += 1
                continue
            # invalidate everything this load overlaps, then record it
            for k in [k for k, v in live.items() if overlap(v[1], rect)]:
                del live[k]
            live[tuple(tp)] = (sig, rect)
            keep.append(inst)
        if removed:
            blk.instructions[:] = keep
    return removed


def _strip_matmul_sem_incs(nc):
    """Every matmul carries a +1 update on the PE progress semaphore;
    each update is a serialized EVT_SEM register write (~26ns) that
    inflates the back-to-back matmul round time. Only the increments
    whose cumulative count is referenced by some wait threshold are
    needed. Keep those (and the last), strip the rest, and remap all
    wait thresholds to the kept-increment numbering."""
    import collections

    mm_types = ("InstMatmult",)
    # gather per-sem: ordered updater list (must be all matmuls), waits
    upd_by_sem = collections.defaultdict(list)  # sem_id -> [inst,...]
    waits_by_sem = collections.defaultdict(list)  # sem_id -> [(inst, wi)]
    bad_sems = set()
    all_insts = []
    for blk in nc.main_func.blocks:
        all_insts.extend(blk.instructions)
    for inst in all_insts:
        si = inst.sync_info
        if si is None:
            continue
        for u in si.on_update:
            if u.sync_type != "semaphore":
                continue
            sid = int(u.id)
            if u.update_mode == "sem-inc" and u.update_reg is None:
                if type(inst).__name__ in mm_types and u.update_value == 1:
                    upd_by_sem[sid].append(inst)
                else:
                    bad_sems.add(sid)
            elif u.update_mode != "sem-set":
                # teardown resets (sem-set) are fine; anything else isn't
                bad_sems.add(sid)
        for wi, w in enumerate(si.on_wait):
            if w.sync_type != "semaphore":
                continue
            sid = int(w.id)
            waits_by_sem[sid].append((inst, wi))
            if w.wait_mode != "sem-ge-imm" or w.wait_reg is not None:
                bad_sems.add(sid)

    stripped = 0
    for sid, updaters in upd_by_sem.items():
        if sid in bad_sems or len(updaters) < 8:
            continue
        referenced = set()
        ok = True
        for inst, wi in waits_by_sem.get(sid, []):
            v = inst.sync_info.on_wait[wi].wait_value
            if v is None or v < 0 or v > len(updaters):
                ok = False
                break
            if v >= 1:
                referenced.add(int(v))
        if not ok:
            continue
        referenced.add(len(updaters))  # keep the final increment
        # map old cumulative count -> new cumulative count
        keep = sorted(referenced)
        newcount = {}
        kept_so_far = 0
        ki = 0
        for oldc in range(1, len(updaters) + 1):
            if ki < len(keep) and keep[ki] == oldc:
                kept_so_far += 1
                ki += 1
            newcount[oldc] = kept_so_far
        # strip updates from non-kept matmuls
        keepset = referenced
        for idx, inst in enumerate(updaters):
            oldc = idx + 1
            if oldc in keepset:
                continue
            si = inst.sync_info
            si.on_update = [
                u
                for u in si.on_update
                if not (u.sync_type == "semaphore" and int(u.id) == sid)
            ]
            stripped += 1
        # remap wait thresholds
        for inst, wi in waits_by_sem.get(sid, []):
            w = inst.sync_info.on_wait[wi]
            if int(w.wait_value) >= 1:
                w.wait_value = newcount[int(w.wait_value)]
    return stripped


def _build_graph():
    nc = bacc.Bacc(
        "TRN2",
        target_bir_lowering=False,
        debug=False,
        enable_asserts=False,
        num_devices=NCORES,
    )
    # conv1 im2col input per group-pair GP: partition 64R + 27a + k holds
    # im2col row k (k = 9c+3dy+dx) of patch (GP,R,g,cg,a); partition
    # 64R+54 = 1.0 (bias row); free dims [g in-pair group, cg pair-sel,
    # 31 y, 31 x].
    xi_d = nc.dram_tensor(
        "xi", [NGP, 2, KIM, 2, 2, HOUT, HOUT], BF16, kind="ExternalInput"
    ).ap()
    w1_d = nc.dram_tensor("w1", [128, 64], BF16, kind="ExternalInput").ap()
    w2_d = nc.dram_tensor("w2", [128, 9 * 128], BF16, kind="ExternalInput").ap()
    b2_d = nc.dram_tensor("b2", [128, 1], F32, kind="ExternalInput").ap()
    wl_d = nc.dram_tensor("wl", [128, 128], BF16, kind="ExternalInput").ap()
    out_d = nc.dram_tensor("out", [128, 8], F32, kind="ExternalOutput").ap()

    RELU = mybir.ActivationFunctionType.Relu
    SQUARE = mybir.ActivationFunctionType.Square
    ADD = mybir.AluOpType.add
    MAX = mybir.AluOpType.max
    SUB = mybir.AluOpType.subtract

    # emission order of group-pairs: ground gp, sat gp interleaved.
    # group-pair gp covers groups (2gp, 2gp+1); ground gps 0-7, sat 8-15.
    GP_ORDER = []
    for i in range(8):
        GP_ORDER.append(i)
        GP_ORDER.append(8 + i)

    with ExitStack() as ctx:
        tc = ctx.enter_context(tile.TileContext(nc))
        const = ctx.enter_context(tc.tile_pool(name="const", bufs=1))
        xpool = ctx.enter_context(tc.tile_pool(name="x", bufs=5))
        hpool = ctx.enter_context(tc.tile_pool(name="h", bufs=3))
        gpool = ctx.enter_context(tc.tile_pool(name="g", bufs=1))
        spool = ctx.enter_context(tc.tile_pool(name="scr", bufs=4))
        pp1 = ctx.enter_context(tc.tile_pool(name="pp1", bufs=4, space="PSUM"))
        pp2 = ctx.enter_context(tc.tile_pool(name="pp2", bufs=4, space="PSUM"))

        xi_tiles: dict = {}

        def issue_dma(i, eng=None):
            # split per (R, g): 4 smaller transfers per pair for lower
            # first-data latency (DMA latency scales with packet size)
            gp = GP_ORDER[i]
            xt = xpool.tile(
                [128, 2, 2, HOUT, HOUT], BF16, tag="xi", name=f"xi_{gp}"
            )
            e = eng if eng is not None else nc.sync
            for R in range(2):
                for g in range(2):
                    e.dma_start(
                        xt[64 * R : 64 * R + KIM, g], xi_d[gp, R, :, g]
                    )
            xi_tiles[gp] = xt

        # --- consts + first group-pairs, spread across the three
        # DMA-capable queues so the first conv1 data lands asap:
        # sync: w1 then gp0's R0; gpsimd: gp0's R1 then gp8;
        # scalar: w2 then gp1, then b2/wl ---
        # scalar queue gets ONLY the three tiny const DMAs -- any big xi
        # issue there jams the eviction FIFO behind DMA-sem recycling
        w1_t = const.tile([128, 64], BF16)
        nc.sync.dma_start(w1_t[:], w1_d)
        w2_t = const.tile([128, 9 * 128], BF16)
        nc.scalar.dma_start(w2_t[:], w2_d)
        b2_t = const.tile([128, 1], F32)
        nc.scalar.dma_start(b2_t[:], b2_d)
        wl_t = const.tile([128, 128], BF16)
        nc.scalar.dma_start(wl_t[:], wl_d)
        issue_dma(0)
        issue_dma(1, eng=nc.gpsimd)
        issue_dma(2, eng=nc.gpsimd)
        issue_dma(3)
        issue_dma(4)

        # gap col layout: for group G (= 2gp+g), jj, q: col 4G+2jj+q;
        # partition 64a+m = patch (G, q, jj?, a)... (cols are summed
        # symmetrically on host, only ground<->sat pairing must match)
        gap = gpool.tile([128, NG * 4], F32)
        res = gpool.tile([128, 8], F32)
        wres = gpool.tile([128, 1], F32)  # warmup sink, never DMA'd

        # PE warm-up burst: keeps the PE busy from the end of the
        # framework prologue until the first conv1 matmul so the HAM
        # clock gate reaches 8/8 as early as possible
        junk = const.tile([128, 448], BF16, name="junk")
        nc.vector.memset(junk[:], 0.5)
        wps = pp1.tile([128, 448], F32, tag="ps1", name="warm_ps")
        for i in range(6):
            nc.tensor.matmul(
                wps[:],
                junk[:, 0:128],
                junk[:],
                start=(i == 0),
                stop=(i == 5),
            )
        wscr = spool.tile([128, 448], F32, tag="wscr")
        nc.scalar.activation(wscr[:], wps[:], SQUARE, accum_out=wres[:, 0:1])

        # greedy ACT/DVE load balancing on estimated busy-ns
        eng_ns = {"act": 0.0, "dve": 0.0}

        def evict_relu(dst, src):
            # conv1 eviction: relu, f32 PSUM -> bf16 SBUF
            if eng_ns["act"] + 630 <= eng_ns["dve"] + 660:
                eng_ns["act"] += 630
                nc.scalar.activation(dst, src, RELU)
            else:
                eng_ns["dve"] += 660
                nc.vector.tensor_scalar_max(dst, src, 0.0)

        def emit_conv1_wave(gp, xt, h1, y0, nr):
            # one y-half of conv1 for a group-pair: 8 matmuls on 4
            # concurrent 64x64 PE tiles (2R x 2cg), 2 g-rounds; weight
            # loads after the first round of a conv1 stretch are deduped
            pss = {}
            for g in range(2):
                for R in range(2):
                    ps = pp1.tile(
                        [128, nr, HOUT], F32, tag="ps1", name=f"c1_{g}{R}"
                    )
                    for cg in range(2):
                        nc.tensor.matmul(
                            ps[64 * cg : 64 * cg + 64, :, :],
                            w1_t[64 * R : 64 * R + KIM, :],
                            xt[
                                64 * R : 64 * R + KIM,
                                g,
                                cg,
                                y0 : y0 + nr,
                                :,
                            ],
                            start=True,
                            stop=True,
                            tile_position=(64 * R, 64 * cg),
                        )
                    pss[(g, R)] = ps
            for g in range(2):
                for R in range(2):
                    evict_relu(
                        h1[:, g, R, y0 : y0 + nr, :], pss[(g, R)][:, :, :]
                    )

        c2_turn = [0]

        def emit_conv2_phase(gp, h1, g):
            # conv2 for one group of a pair: all 9 offsets, then evict.
            # Using only 2 of the 4 pp2 slots per phase leaves the pool
            # double-buffered across phases, so o=0 never stalls on
            # the previous phase's evictions.
            ps2s = {
                jj: pp2.tile(
                    [128, 2, COUT * COUT], F32, tag="ps2", name=f"ps2_{g}{jj}"
                )
                for jj in range(2)
            }
            for o in range(9):
                dy, dx = o // 3, o % 3
                for jj in range(2):
                    p0 = 64 * jj
                    nc.tensor.matmul(
                        ps2s[jj][:],
                        w2_t[p0 : p0 + 64, 128 * o : 128 * o + 128],
                        h1[
                            p0 : p0 + 64,
                            g,
                            :,
                            dy : dy + 29 : 2,
                            dx : dx + 29 : 2,
                        ],
                        start=(o == 0),
                        stop=(o == 8),
                        tile_position=(p0, 0),
                    )
            # eviction: relu(x + b2) then GAP sum into 2 gap columns.
            # NOTE: DVE accum_out is broken on TRN2 hardware (and
            # clobbers op1) -- only ACT may use accum_out. Plans:
            #  a) ACT relu->bf16 scratch + DVE segmented reduce
            #  b) DVE relu->bf16 scratch + DVE segmented reduce
            #  c) ACT in-place relu+bias+accum per q (no scratch)
            G = 2 * gp + g
            for jj in range(2):
                src = ps2s[jj]
                col = 4 * G + 2 * jj
                costs = {
                    "a": max(eng_ns["act"] + 630, eng_ns["dve"] + 613),
                    "b": eng_ns["dve"] + 1258,
                    "c": eng_ns["act"] + 1834,
                }
                plan = min(costs, key=costs.get)
                if plan == "c":
                    eng_ns["act"] += 1374
                    for q in range(2):
                        nc.scalar.activation(
                            src[:, q, :],
                            src[:, q, :],
                            RELU,
                            bias=b2_t[:],
                            accum_out=gap[:, col + q : col + q + 1],
                        )
                    continue
                scr = spool.tile(
                    [128, 2, COUT * COUT], BF16, tag="scr2", name="scr2"
                )
                if plan == "a":
                    eng_ns["act"] += 630
                    nc.scalar.activation(scr[:], src[:], RELU, bias=b2_t[:])
                else:
                    eng_ns["dve"] += 645
                    nc.vector.tensor_scalar(
                        scr[:], src[:], b2_t[:], 0.0, op0=ADD, op1=MAX
                    )
                eng_ns["dve"] += 613
                nc.vector.tensor_reduce(
                    gap[:, col : col + 2],
                    scr[:],
                    axis=mybir.AxisListType.X,
                    op=ADD,
                )

        def emit_chunk_diff(c):
            # feature diffs for 16 ground cols [16c, 16c+16) paired with
            # sat cols [64+16c, 80+16c), on GpSimd (SBUF-only engine)
            c0 = 16 * c
            dg = spool.tile([128, 16], F32, tag="dg", name=f"dg_{c}")
            dgb = spool.tile([128, 16], BF16, tag="dgb", name=f"dgb_{c}")
            nc.gpsimd.tensor_tensor(
                dg[:], gap[:, c0 : c0 + 16], gap[:, 64 + c0 : 80 + c0], op=SUB
            )
            nc.gpsimd.tensor_copy(dgb[:], dg[:])
            return dgb

        def emit_chunk_mm(c, dgb):
            # linear + square for a finished chunk (deferred so the PE
            # queue never waits on the GpSimd diff chain).
            for jj in range(2):
                p0 = 64 * jj
                ps3 = pp2.tile([128, 16], F32, tag="ps2", name=f"ps3_{c}{jj}")
                nc.tensor.matmul(
                    ps3[:],
                    wl_t[p0 : p0 + 64, :],
                    dgb[p0 : p0 + 64, :],
                    start=True,
                    stop=True,
                    tile_position=(p0, 0),
                )
                scr3 = spool.tile(
                    [128, 16], F32, tag="scr3", name=f"scr3_{c}{jj}"
                )
                nc.scalar.activation(
                    scr3[:], ps3[:], SQUARE, accum_out=res[:, 2 * c + jj : 2 * c + jj + 1]
                )
            # 2 unused pad allocations keep the 4-slot pp2 rotation
            # parity; they land on the in-flight g0 slots but are never
            # written or read, so they cannot stall anything
            for _pad in range(2):
                pp2.tile([128, 16], F32, tag="ps2", name=f"pad_{c}{_pad}")

        # software-pipelined emission: conv1 y-waves of pair i interleave
        # with the two conv2 group-phases of pair i-2; endgame chunk
        # after every 2nd sat group-pair
        pending = []
        done_sat = [0]
        chunk_q = []  # (c, dgb) whose PE part is deferred

        def flush_chunks(last=False):
            while chunk_q:
                c, dgb = chunk_q.pop(0)
                emit_chunk_mm(c, dgb)
                if c == 1:
                    nc.sync.dma_start(out_d[:, 0:4], res[:, 0:4])
                if c == 3 and last:
                    nc.sync.dma_start(out_d[:, 4:8], res[:, 4:8])

        def after_conv2(gp2):
            if gp2 >= 8:
                done_sat[0] += 1
                if done_sat[0] % 2 == 0:
                    c = done_sat[0] // 2 - 1
                    chunk_q.append((c, emit_chunk_diff(c)))

        for i in range(NGP):
            gp = GP_ORDER[i]
            xt = xi_tiles.pop(gp)
            h1 = hpool.tile(
                [128, 2, 2, HOUT, HOUT], BF16, tag="h1", name=f"h1_{gp}"
            )
            # head schedule: conv2(gp0) already at iteration 1 (depth 1)
            # so the PE never sits behind the DMA ramp; depth restored
            # to 2 by the conv2-free iteration 2
            take = (i == 1) or (i >= 3)
            work = pending.pop(0) if (take and pending) else None
            emit_conv1_wave(gp, xt, h1, 0, 16)
            if i + 5 < NGP:
                issue_dma(i + 5, eng=(nc.gpsimd if i % 2 else nc.sync))
            if work is not None:
                emit_conv2_phase(work[0], work[1], 0)
                flush_chunks()
            emit_conv1_wave(gp, xt, h1, 16, 15)
            pending.append((gp, h1))
            if work is not None:
                emit_conv2_phase(work[0], work[1], 1)
                after_conv2(work[0])
        while pending:
            gp2, h2 = pending.pop(0)
            emit_conv2_phase(gp2, h2, 0)
            flush_chunks()
            emit_conv2_phase(gp2, h2, 1)
            after_conv2(gp2)
        flush_chunks(last=True)

    ndup = _dedupe_ldweights(nc)
    nsem = _strip_matmul_sem_incs(nc)
    print(
        f"[kernel] deduped {ndup} LDWEIGHTS, stripped {nsem} sem incs",
        file=sys.stderr,
    )
    nc.compile()
    return nc


def _prep_weights(w1, b1, w2, b2, wl):
    # conv1 im2col weights: [64R + 27a + (9c+3dy+3?dx), 32a+m]
    w1i = np.zeros((128, 64), np.float32)
    for a in range(2):
        for c in range(3):
            for dy in range(3):
                for dx in range(3):
                    w1i[27 * a + 9 * c + 3 * dy + dx, 32 * a : 32 * a + 32] = w1[
                        :, c, dy, dx
                    ]
        w1i[54, 32 * a : 32 * a + 32] = b1
    w1i[64:119] = w1i[0:55]

    w2blk = np.zeros((128, 9, 128), np.float32)
    for jj in range(2):
        for j in range(2):
            for c in range(32):
                for o in range(9):
                    dy, dx = o // 3, o % 3
                    w2blk[64 * jj + 32 * j + c, o, 64 * j : 64 * j + 64] = w2[
                        :, c, dy, dx
                    ]
    b2q = np.tile(b2, 2)[:, None].astype(np.float32)  # unscaled
    wlrep = np.zeros((128, 128), np.float32)
    wlrep[0:64] = wl.T
    wlrep[64:128] = wl.T
    return (
        w1i.astype(NPBF16),
        w2blk.reshape(128, 9 * 128).astype(NPBF16),
        np.ascontiguousarray(b2q),
        wlrep.astype(NPBF16),
    )


def _crop_all(images, kps):
    # images [B,3,H,W] f32; kps [NKP,2] normalized -> patches [NKP,B,3,P,P]
    hw = images.shape[-1]
    px = kps.astype(np.float32) * np.float32(hw)
    starts = np.clip(np.floor(px).astype(np.int32) - SIGMA, 0, hw - PATCH)
    out = np.empty((kps.shape[0], images.shape[0], 3, PATCH, PATCH), np.float32)
    for n in range(kps.shape[0]):
        x, y = int(starts[n, 0]), int(starts[n, 1])
        out[n] = images[:, :, y : y + PATCH, x : x + PATCH]
    return out


def _im2col_groups(pat):
    # pat [128,3,33,33] (one set for one core) -> [8, 2, 55, 2, 2, 31, 31]
    # (gp, R, im2col row (27a+9c+3dy+dx | 54=ones), g in-pair, cg, y, x)
    sw = np.lib.stride_tricks.sliding_window_view(pat, (HOUT, HOUT), axis=(2, 3))
    # sw[n, c, dy, dx, y, x] = pat[n, c, dy+y, dx+x]
    sw = sw.reshape(8, 2, 2, 2, 2, 27, HOUT, HOUT)  # (gp, g, R, cg, a, k, y, x)
    out = np.empty((8, 2, KIM, 2, 2, HOUT, HOUT), np.float32)
    # target row = 27a + k; dims (gp, R, a, k, g, cg, y, x)
    out[:, :, :54] = sw.transpose(0, 2, 4, 5, 1, 3, 6, 7).reshape(
        8, 2, 54, 2, 2, HOUT, HOUT
    )
    out[:, :, 54] = 1.0
    return out


def _make_in_maps(np_inputs):
    images_ground = np.asarray(np_inputs["images_ground"], np.float32)
    images_satellite = np.asarray(np_inputs["images_satellite"], np.float32)
    kg = np.asarray(np_inputs["keypoints_ground"], np.float32).reshape(-1, 2)
    ks = np.asarray(np_inputs["keypoints_satellite"], np.float32).reshape(-1, 2)
    w1 = np.asarray(np_inputs["w1"], np.float32)
    b1 = np.asarray(np_inputs["b1"], np.float32)
    w2 = np.asarray(np_inputs["w2"], np.float32)
    b2 = np.asarray(np_inputs["b2"], np.float32)
    wl = np.asarray(np_inputs["wl"], np.float32)

    pg = _crop_all(images_ground, kg)  # [256,4,3,33,33]
    ps = _crop_all(images_satellite, ks)
    w1i, w2blk, b2q, wlrep = _prep_weights(w1, b1, w2, b2, wl)

    in_maps = []
    for i in range(NCORES):
        sl = slice(i * KPC, (i + 1) * KPC)
        patg = pg[sl].reshape(NPATCH, 3, PATCH, PATCH)
        pats = ps[sl].reshape(NPATCH, 3, PATCH, PATCH)
        xi = np.concatenate(
            [_im2col_groups(patg), _im2col_groups(pats)], axis=0
        ).astype(NPBF16)
        in_maps.append(dict(xi=xi, w1=w1i, w2=w2blk, b2=b2q, wl=wlrep))
    return in_maps


def kernel(**inputs):
    in_maps = _make_in_maps(inputs)

    if "nc" not in _CACHE:
        _CACHE["nc"] = _build_graph()
    nc = _CACHE["nc"]

    results = bass_utils.run_bass_kernel_spmd(
        nc, in_maps, core_ids=list(range(NCORES))
    )
    total = np.float64(0.0)
    for r in results.results:
        total += np.asarray(r["out"], np.float64).sum()
    mse = total / (NKP * B * 128 * (COUT * COUT) ** 2)
    return np.asarray(mse, np.float32)


if __name__ == "__main__":
    rng = np.random.default_rng(0)
    ins = dict(
        images_ground=rng.standard_normal((B, 3, H, H)).astype(np.float32),
        images_satellite=rng.standard_normal((B, 3, H, H)).astype(np.float32),
        keypoints_ground=(0.2 + 0.6 * rng.random((B, K, 2))).astype(np.float32),
        keypoints_satellite=(0.2 + 0.6 * rng.random((B, K, 2))).astype(np.float32),
        w1=(rng.standard_normal((32, 3, 3, 3)) * 0.1).astype(np.float32),
        b1=np.zeros(32, np.float32),
        w2=(rng.standard_normal((64, 32, 3, 3)) * 0.05).astype(np.float32),
        b2=np.zeros(64, np.float32),
        wl=(rng.standard_normal((128, 64)) * 0.1).astype(np.float32),
        bl=np.zeros(128, np.float32),
        num_samples=K,
    )
    print("kernel out:", kernel(**ins))


# revision 40
# speedup vs baseline: 1.0351x; 1.0351x over previous
"""Trainium2 Bass kernel for nn_AppearanceLoss (keypoint patch CNN MSE).

Host: crops 33x33 patches at keypoint locations, builds full im2col
(27 rows = 3c x 3dy x 3dx per patch) so conv1 is a single-shot matmul,
shards 256 keypoints across 8 NeuronCores.

Device (v2): group-PAIR structure (2 groups = 16 patches per iteration,
ground/satellite pairs interleaved so the MSE endgame chunks through the
kernel). conv1 = 64x64 PE-tiling, 4 concurrent tiles; conv2 =
offset-accumulated K=64 block-diag matmuls, one weight load per
(offset, row-half) serving both groups of the pair (a post-build pass
deletes redundant LDWEIGHTS). conv2 eviction = relu+bias to bf16
scratch on ACT/DVE + segmented GAP reduce on GpSimd. Linear on feature
diffs computed in 4 chunks during steady state; output DMA split in two.
Host sums 8 per-core partials into the scalar MSE.
"""

import sys

sys.path.insert(0, "/opt/trn_rl_repo")

from contextlib import ExitStack

import ml_dtypes
import numpy as np

import concourse.bass as bass  # noqa: F401
import concourse.tile as tile
from concourse import bacc, bass_utils, mybir

SIGMA = 16
PATCH = 33  # 2*SIGMA+1
HOUT = 31  # conv1 valid output: 33-3+1
COUT = 15  # conv2 stride-2 valid output: (31-3)//2+1
B, K, H = 4, 64, 256
NCORES = 8
NKP = B * K  # 256 keypoints total
KPC = NKP // NCORES  # 32 keypoints per core
NPATCH = KPC * B  # 128 patches per core per set
NG = 32  # groups of 8 patches (16 ground + 16 sat)
NGP = NG // 2  # 16 group-pairs
KIM = 55  # conv1 im2col rows per pair: 2*27 + ones
BF16 = mybir.dt.bfloat16
F32 = mybir.dt.float32
NPBF16 = ml_dtypes.bfloat16

_CACHE: dict = {}


def _dedupe_ldweights(nc):
    """Remove InstLdweights that reload weights already resident in the
    same PE tile position (identical access pattern, no intervening
    overlapping load). Waits on a removed load move to the next
    instruction (its matmul); loads with updates are kept."""
    removed = 0
    for blk in nc.main_func.blocks:
        referenced = set()
        for inst in blk.instructions:
            try:
                for name, _ in inst.dependency_edges():
                    referenced.add(name)
            except Exception:
                pass
        live = {}  # tile_position -> (signature, rect)

        def overlap(a, b):
            return a[0] < b[1] and b[0] < a[1] and a[2] < b[3] and b[2] < a[3]

        insts = blk.instructions
        keep = []
        for idx, inst in enumerate(insts):
            tname = type(inst).__name__
            if tname != "InstLdweights":
                keep.append(inst)
                continue
            tp = inst.tile_position
            ts = inst.tile_size
            if tp is None or ts is None:
                live.clear()
                keep.append(inst)
                continue
            rect = (tp[0], tp[0] + ts[0], tp[1], tp[1] + ts[1])
            sig = (
                tuple(tp),
                tuple(ts),
                inst.perf_mode,
                inst.is_transpose,
                str(inst.ins[0]),
            )
            si = inst.sync_info
            has_update = si is not None and len(si.on_update) > 0
            prev = live.get(tuple(tp))
            if (
                prev is not None
                and prev[0] == sig
                and not has_update
                and inst.name not in referenced
            ):
                # redundant: same weights already loaded at this position
                waits = list(si.on_wait) if si is not None else []
                if waits:
                    # move waits onto the next instruction (its matmul)
                    nxt = insts[idx + 1]
                    nsi = nxt.sync_info
                    if nsi is None:
                        nxt.sync_info = mybir.SyncInfo(
                            on_wait=waits, on_update=[]
                        )
                    else:
                        nsi.on_wait = list(nsi.on_wait) + waits
                removed += 1
                continue
            # invalidate everything this load overlaps, then record it
            for k in [k for k, v in live.items() if overlap(v[1], rect)]:
                del live[k]
            live[tuple(tp)] = (sig, rect)
            keep.append(inst)
        if removed:
            blk.instructions[:] = keep
    return removed


def _strip_matmul_sem_incs(nc):
    """Every matmul carries a +1 update on the PE progress semaphore;
    each update is a serialized EVT_SEM register write (~26ns) that
    inflates the back-to-back matmul round time. Only the increments
    whose cumulative count is referenced by some wait threshold are
    needed. Keep those (and the last), strip the rest, and remap all
    wait thresholds to the kept-increment numbering."""
    import collections

    mm_types = ("InstMatmult",)
    # gather per-sem: ordered updater list (must be all matmuls), waits
    upd_by_sem = collections.defaultdict(list)  # sem_id -> [inst,...]
    waits_by_sem = collections.defaultdict(list)  # sem_id -> [(inst, wi)]
    bad_sems = set()
    all_insts = []
    for blk in nc.main_func.blocks:
        all_insts.extend(blk.instructions)
    for inst in all_insts:
        si = inst.sync_info
        if si is None:
            continue
        for u in si.on_update:
            if u.sync_type != "semaphore":
                continue
            sid = int(u.id)
            if u.update_mode == "sem-inc" and u.update_reg is None:
                if type(inst).__name__ in mm_types and u.update_value == 1:
                    upd_by_sem[sid].append(inst)
                else:
                    bad_sems.add(sid)
            elif u.update_mode != "sem-set":
                # teardown resets (sem-set) are fine; anything else isn't
                bad_sems.add(sid)
        for wi, w in enumerate(si.on_wait):
            if w.sync_type != "semaphore":
                continue
            sid = int(w.id)
            waits_by_sem[sid].append((inst, wi))
            if w.wait_mode != "sem-ge-imm" or w.wait_reg is not None:
                bad_sems.add(sid)

    stripped = 0
    for sid, updaters in upd_by_sem.items():
        if sid in bad_sems or len(updaters) < 8:
            continue
        referenced = set()
        ok = True
        for inst, wi in waits_by_sem.get(sid, []):
            v = inst.sync_info.on_wait[wi].wait_value
            if v is None or v < 0 or v > len(updaters):
                ok = False
                break
            if v >= 1:
                referenced.add(int(v))
        if not ok:
            continue
        referenced.add(len(updaters))  # keep the final increment
        # map old cumulative count -> new cumulative count
        keep = sorted(referenced)
        newcount = {}
        kept_so_far = 0
        ki = 0
        for oldc in range(1, len(updaters) + 1):
            if ki < len(keep) and keep[ki] == oldc:
                kept_so_far += 1
                ki += 1
            newcount[oldc] = kept_so_far
        # strip updates from non-kept matmuls
        keepset = referenced
        for idx, inst in enumerate(updaters):
            oldc = idx + 1
            if oldc in keepset:
                continue
            si = inst.sync_info
            si.on_update = [
                u
                for u in si.on_update
                if not (u.sync_type == "semaphore" and int(u.id) == sid)
            ]
            stripped += 1
        # remap wait thresholds
        for inst, wi in waits_by_sem.get(sid, []):
            w = inst.sync_info.on_wait[wi]
            if int(w.wait_value) >= 1:
                w.wait_value = newcount[int(w.wait_value)]
    return stripped


def _build_graph():
    nc = bacc.Bacc(
        "TRN2",
        target_bir_lowering=False,
        debug=False,
        enable_asserts=False,
        num_devices=NCORES,
    )
    # conv1 im2col input per group-pair GP: partition 64R + 27a + k holds
    # im2col row k (k = 9c+3dy+dx) of patch (GP,R,g,cg,a); partition
    # 64R+54 = 1.0 (bias row); free dims [g in-pair group, cg pair-sel,
    # 31 y, 31 x].
    xi_d = nc.dram_tensor(
        "xi", [NGP, 2, KIM, 2, 2, HOUT, HOUT], BF16, kind="ExternalInput"
    ).ap()
    w1_d = nc.dram_tensor("w1", [128, 64], BF16, kind="ExternalInput").ap()
    w2_d = nc.dram_tensor("w2", [128, 9 * 128], BF16, kind="ExternalInput").ap()
    b2_d = nc.dram_tensor("b2", [128, 1], F32, kind="ExternalInput").ap()
    wl_d = nc.dram_tensor("wl", [128, 128], BF16, kind="ExternalInput").ap()
    out_d = nc.dram_tensor("out", [128, 8], F32, kind="ExternalOutput").ap()

    RELU = mybir.ActivationFunctionType.Relu
    SQUARE = mybir.ActivationFunctionType.Square
    ADD = mybir.AluOpType.add
    MAX = mybir.AluOpType.max
    SUB = mybir.AluOpType.subtract

    # emission order of group-pairs: ground gp, sat gp interleaved.
    # group-pair gp covers groups (2gp, 2gp+1); ground gps 0-7, sat 8-15.
    GP_ORDER = []
    for i in range(8):
        GP_ORDER.append(i)
        GP_ORDER.append(8 + i)

    with ExitStack() as ctx:
        tc = ctx.enter_context(tile.TileContext(nc))
        const = ctx.enter_context(tc.tile_pool(name="const", bufs=1))
        xpool = ctx.enter_context(tc.tile_pool(name="x", bufs=4))
        hpool = ctx.enter_context(tc.tile_pool(name="h", bufs=3))
        gpool = ctx.enter_context(tc.tile_pool(name="g", bufs=1))
        spool = ctx.enter_context(tc.tile_pool(name="scr", bufs=4))
        pp1 = ctx.enter_context(tc.tile_pool(name="pp1", bufs=4, space="PSUM"))
        pp2 = ctx.enter_context(tc.tile_pool(name="pp2", bufs=4, space="PSUM"))

        xi_tiles: dict = {}

        def issue_dma(i):
            # split per (R, g): 4 transfers per pair, R0 pieces on the
            # sync queue and R1 on gpsimd so both DMA rings pull
            gp = GP_ORDER[i]
            xt = xpool.tile(
                [128, 2, 2, HOUT, HOUT], BF16, tag="xi", name=f"xi_{gp}"
            )
            for R, e in ((0, nc.sync), (1, nc.gpsimd)):
                for g in range(2):
                    e.dma_start(
                        xt[64 * R : 64 * R + KIM, g], xi_d[gp, R, :, g]
                    )
            xi_tiles[gp] = xt

        # --- consts + first group-pairs, spread across the three
        # DMA-capable queues so the first conv1 data lands asap:
        # sync: w1 then gp0's R0; gpsimd: gp0's R1 then gp8;
        # scalar: w2 then gp1, then b2/wl ---
        # scalar queue gets ONLY the three tiny const DMAs -- any big xi
        # issue there jams the eviction FIFO behind DMA-sem recycling.
        # xi gp0 is issued ALONE first so it monopolizes the DMA fabric
        # (the engines round-robin; co-issued pairs all finish late).
        w1_t = const.tile([128, 64], BF16)
        nc.sync.dma_start(w1_t[:], w1_d)
        issue_dma(0)
        w2_t = const.tile([128, 9 * 128], BF16)
        nc.scalar.dma_start(w2_t[:], w2_d)
        b2_t = const.tile([128, 1], F32)
        nc.scalar.dma_start(b2_t[:], b2_d)
        wl_t = const.tile([128, 128], BF16)
        nc.scalar.dma_start(wl_t[:], wl_d)
        issue_dma(1)
        issue_dma(2)

        # gap col layout: for group G (= 2gp+g), jj, q: col 4G+2jj+q;
        # partition 64a+m = patch (G, q, jj?, a)... (cols are summed
        # symmetrically on host, only ground<->sat pairing must match)
        gap = gpool.tile([128, NG * 4], F32)
        res = gpool.tile([128, 8], F32)
        wres = gpool.tile([128, 1], F32)  # warmup sink, never DMA'd

        # PE warm-up burst: keeps the PE busy from the end of the
        # framework prologue until the first conv1 matmul so the HAM
        # clock gate reaches 8/8 as early as possible
        junk = const.tile([128, 320], BF16, name="junk")
        nc.vector.memset(junk[:], 0.5)
        wps = pp1.tile([128, 320], F32, tag="ps1", name="warm_ps")
        for i in range(5):
            nc.tensor.matmul(
                wps[:],
                junk[:, 0:128],
                junk[:],
                start=(i == 0),
                stop=(i == 4),
            )
        wscr = spool.tile([128, 320], F32, tag="wscr")
        nc.scalar.activation(wscr[:], wps[:], SQUARE, accum_out=wres[:, 0:1])

        # greedy ACT/DVE load balancing on estimated busy-ns
        eng_ns = {"act": 0.0, "dve": 0.0}

        def evict_relu(dst, src):
            # conv1 eviction: relu, f32 PSUM -> bf16 SBUF
            if eng_ns["act"] + 630 <= eng_ns["dve"] + 660:
                eng_ns["act"] += 630
                nc.scalar.activation(dst, src, RELU)
            else:
                eng_ns["dve"] += 660
                nc.vector.tensor_scalar_max(dst, src, 0.0)

        def emit_conv1_wave(gp, xt, h1, y0, nr):
            # one y-half of conv1 for a group-pair: 8 matmuls on 4
            # concurrent 64x64 PE tiles (2R x 2cg), 2 g-rounds; weight
            # loads after the first round of a conv1 stretch are deduped
            pss = {}
            for g in range(2):
                for R in range(2):
                    ps = pp1.tile(
                        [128, nr, HOUT], F32, tag="ps1", name=f"c1_{g}{R}"
                    )
                    for cg in range(2):
                        nc.tensor.matmul(
                            ps[64 * cg : 64 * cg + 64, :, :],
                            w1_t[64 * R : 64 * R + KIM, :],
                            xt[
                                64 * R : 64 * R + KIM,
                                g,
                                cg,
                                y0 : y0 + nr,
                                :,
                            ],
                            start=True,
                            stop=True,
                            tile_position=(64 * R, 64 * cg),
                        )
                    pss[(g, R)] = ps
            for g in range(2):
                for R in range(2):
                    evict_relu(
                        h1[:, g, R, y0 : y0 + nr, :], pss[(g, R)][:, :, :]
                    )

        c2_turn = [0]

        def emit_conv2_phase(gp, h1, g):
            # conv2 for one group of a pair: all 9 offsets, then evict.
            # Using only 2 of the 4 pp2 slots per phase leaves the pool
            # double-buffered across phases, so o=0 never stalls on
            # the previous phase's evictions.
            ps2s = {
                jj: pp2.tile(
                    [128, 2, COUT * COUT], F32, tag="ps2", name=f"ps2_{g}{jj}"
                )
                for jj in range(2)
            }
            for o in range(9):
                dy, dx = o // 3, o % 3
                for jj in range(2):
                    p0 = 64 * jj
                    nc.tensor.matmul(
                        ps2s[jj][:],
                        w2_t[p0 : p0 + 64, 128 * o : 128 * o + 128],
                        h1[
                            p0 : p0 + 64,
                            g,
                            :,
                            dy : dy + 29 : 2,
                            dx : dx + 29 : 2,
                        ],
                        start=(o == 0),
                        stop=(o == 8),
                        tile_position=(p0, 0),
                    )
            # eviction: relu(x + b2) then GAP sum into 2 gap columns.
            # NOTE: DVE accum_out is broken on TRN2 hardware (and
            # clobbers op1) -- only ACT may use accum_out. Plans:
            #  a) ACT relu->bf16 scratch + DVE segmented reduce
            #  b) DVE relu->bf16 scratch + DVE segmented reduce
            #  c) ACT in-place relu+bias+accum per q (no scratch)
            G = 2 * gp + g
            for jj in range(2):
                src = ps2s[jj]
                col = 4 * G + 2 * jj
                costs = {
                    "a": max(eng_ns["act"] + 630, eng_ns["dve"] + 613),
                    "b": eng_ns["dve"] + 1258,
                    "c": eng_ns["act"] + 1834,
                }
                plan = min(costs, key=costs.get)
                if plan == "c":
                    eng_ns["act"] += 1374
                    for q in range(2):
                        nc.scalar.activation(
                            src[:, q, :],
                            src[:, q, :],
                            RELU,
                            bias=b2_t[:],
                            accum_out=gap[:, col + q : col + q + 1],
                        )
                    continue
                scr = spool.tile(
                    [128, 2, COUT * COUT], BF16, tag="scr2", name="scr2"
                )
                if plan == "a":
                    eng_ns["act"] += 630
                    nc.scalar.activation(scr[:], src[:], RELU, bias=b2_t[:])
                else:
                    eng_ns["dve"] += 645
                    nc.vector.tensor_scalar(
                        scr[:], src[:], b2_t[:], 0.0, op0=ADD, op1=MAX
                    )
                eng_ns["dve"] += 613
                nc.vector.tensor_reduce(
                    gap[:, col : col + 2],
                    scr[:],
                    axis=mybir.AxisListType.X,
                    op=ADD,
                )

        def emit_chunk_diff(c):
            # feature diffs for 16 ground cols [16c, 16c+16) paired with
            # sat cols [64+16c, 80+16c), on GpSimd (SBUF-only engine)
            c0 = 16 * c
            dg = spool.tile([128, 16], F32, tag="dg", name=f"dg_{c}")
            dgb = spool.tile([128, 16], BF16, tag="dgb", name=f"dgb_{c}")
            nc.gpsimd.tensor_tensor(
                dg[:], gap[:, c0 : c0 + 16], gap[:, 64 + c0 : 80 + c0], op=SUB
            )
            nc.gpsimd.tensor_copy(dgb[:], dg[:])
            return dgb

        def emit_chunk_mm(c, dgb):
            # linear + square for a finished chunk (deferred so the PE
            # queue never waits on the GpSimd diff chain).
            for jj in range(2):
                p0 = 64 * jj
                ps3 = pp2.tile([128, 16], F32, tag="ps2", name=f"ps3_{c}{jj}")
                nc.tensor.matmul(
                    ps3[:],
                    wl_t[p0 : p0 + 64, :],
                    dgb[p0 : p0 + 64, :],
                    start=True,
                    stop=True,
                    tile_position=(p0, 0),
                )
                scr3 = spool.tile(
                    [128, 16], F32, tag="scr3", name=f"scr3_{c}{jj}"
                )
                nc.scalar.activation(
                    scr3[:], ps3[:], SQUARE, accum_out=res[:, 2 * c + jj : 2 * c + jj + 1]
                )
            # 2 unused pad allocations keep the 4-slot pp2 rotation
            # parity; they land on the in-flight g0 slots but are never
            # written or read, so they cannot stall anything
            for _pad in range(2):
                pp2.tile([128, 16], F32, tag="ps2", name=f"pad_{c}{_pad}")

        # software-pipelined emission: conv1 y-waves of pair i interleave
        # with the two conv2 group-phases of pair i-2; endgame chunk
        # after every 2nd sat group-pair
        pending = []
        done_sat = [0]
        chunk_q = []  # (c, dgb) whose PE part is deferred

        def flush_chunks(last=False):
            while chunk_q:
                c, dgb = chunk_q.pop(0)
                emit_chunk_mm(c, dgb)
                if c == 1:
                    nc.sync.dma_start(out_d[:, 0:4], res[:, 0:4])
                if c == 3 and last:
                    nc.sync.dma_start(out_d[:, 4:8], res[:, 4:8])

        def after_conv2(gp2):
            if gp2 >= 8:
                done_sat[0] += 1
                if done_sat[0] % 2 == 0:
                    c = done_sat[0] // 2 - 1
                    chunk_q.append((c, emit_chunk_diff(c)))

        for i in range(NGP):
            gp = GP_ORDER[i]
            xt = xi_tiles.pop(gp)
            h1 = hpool.tile(
                [128, 2, 2, HOUT, HOUT], BF16, tag="h1", name=f"h1_{gp}"
            )
            # head schedule: conv2(gp0) already at iteration 1 (depth 1)
            # so the PE never sits behind the DMA ramp; depth restored
            # to 2 by the conv2-free iteration 2
            take = (i == 1) or (i >= 3)
            work = pending.pop(0) if (take and pending) else None
            emit_conv1_wave(gp, xt, h1, 0, 16)
            if i + 3 < NGP:
                issue_dma(i + 3)
            if work is not None:
                emit_conv2_phase(work[0], work[1], 0)
                flush_chunks()
            emit_conv1_wave(gp, xt, h1, 16, 15)
            pending.append((gp, h1))
            if work is not None:
                emit_conv2_phase(work[0], work[1], 1)
                after_conv2(work[0])
        while pending:
            gp2, h2 = pending.pop(0)
            emit_conv2_phase(gp2, h2, 0)
            flush_chunks()
            emit_conv2_phase(gp2, h2, 1)
            after_conv2(gp2)
        flush_chunks(last=True)

    ndup = _dedupe_ldweights(nc)
    nsem = _strip_matmul_sem_incs(nc)
    print(
        f"[kernel] deduped {ndup} LDWEIGHTS, stripped {nsem} sem incs",
        file=sys.stderr,
    )
    nc.compile()
    return nc


def _prep_weights(w1, b1, w2, b2, wl):
    # conv1 im2col weights: [64R + 27a + (9c+3dy+3?dx), 32a+m]
    w1i = np.zeros((128, 64), np.float32)
    for a in range(2):
        for c in range(3):
            for dy in range(3):
                for dx in range(3):
                    w1i[27 * a + 9 * c + 3 * dy + dx, 32 * a : 32 * a + 32] = w1[
                        :, c, dy, dx
                    ]
        w1i[54, 32 * a : 32 * a + 32] = b1
    w1i[64:119] = w1i[0:55]

    w2blk = np.zeros((128, 9, 128), np.float32)
    for jj in range(2):
        for j in range(2):
            for c in range(32):
                for o in range(9):
                    dy, dx = o // 3, o % 3
                    w2blk[64 * jj + 32 * j + c, o, 64 * j : 64 * j + 64] = w2[
                        :, c, dy, dx
                    ]
    b2q = np.tile(b2, 2)[:, None].astype(np.float32)  # unscaled
    wlrep = np.zeros((128, 128), np.float32)
    wlrep[0:64] = wl.T
    wlrep[64:128] = wl.T
    return (
        w1i.astype(NPBF16),
        w2blk.reshape(128, 9 * 128).astype(NPBF16),
        np.ascontiguousarray(b2q),
        wlrep.astype(NPBF16),
    )


def _crop_all(images, kps):
    # images [B,3,H,W] f32; kps [NKP,2] normalized -> patches [NKP,B,3,P,P]
    hw = images.shape[-1]
    px = kps.astype(np.float32) * np.float32(hw)
    starts = np.clip(np.floor(px).astype(np.int32) - SIGMA, 0, hw - PATCH)
    out = np.empty((kps.shape[0], images.shape[0], 3, PATCH, PATCH), np.float32)
    for n in range(kps.shape[0]):
        x, y = int(starts[n, 0]), int(starts[n, 1])
        out[n] = images[:, :, y : y + PATCH, x : x + PATCH]
    return out


def _im2col_groups(pat):
    # pat [128,3,33,33] (one set for one core) -> [8, 2, 55, 2, 2, 31, 31]
    # (gp, R, im2col row (27a+9c+3dy+dx | 54=ones), g in-pair, cg, y, x)
    sw = np.lib.stride_tricks.sliding_window_view(pat, (HOUT, HOUT), axis=(2, 3))
    # sw[n, c, dy, dx, y, x] = pat[n, c, dy+y, dx+x]
    sw = sw.reshape(8, 2, 2, 2, 2, 27, HOUT, HOUT)  # (gp, g, R, cg, a, k, y, x)
    out = np.empty((8, 2, KIM, 2, 2, HOUT, HOUT), np.float32)
    # target row = 27a + k; dims (gp, R, a, k, g, cg, y, x)
    out[:, :, :54] = sw.transpose(0, 2, 4, 5, 1, 3, 6, 7).reshape(
        8, 2, 54, 2, 2, HOUT, HOUT
    )
    out[:, :, 54] = 1.0
    return out


def _make_in_maps(np_inputs):
    images_ground = np.asarray(np_inputs["images_ground"], np.float32)
    images_satellite = np.asarray(np_inputs["images_satellite"], np.float32)
    kg = np.asarray(np_inputs["keypoints_ground"], np.float32).reshape(-1, 2)
    ks = np.asarray(np_inputs["keypoints_satellite"], np.float32).reshape(-1, 2)
    w1 = np.asarray(np_inputs["w1"], np.float32)
    b1 = np.asarray(np_inputs["b1"], np.float32)
    w2 = np.asarray(np_inputs["w2"], np.float32)
    b2 = np.asarray(np_inputs["b2"], np.float32)
    wl = np.asarray(np_inputs["wl"], np.float32)

    pg = _crop_all(images_ground, kg)  # [256,4,3,33,33]
    ps = _crop_all(images_satellite, ks)
    w1i, w2blk, b2q, wlrep = _prep_weights(w1, b1, w2, b2, wl)

    in_maps = []
    for i in range(NCORES):
        sl = slice(i * KPC, (i + 1) * KPC)
        patg = pg[sl].reshape(NPATCH, 3, PATCH, PATCH)
        pats = ps[sl].reshape(NPATCH, 3, PATCH, PATCH)
        xi = np.concatenate(
            [_im2col_groups(patg), _im2col_groups(pats)], axis=0
        ).astype(NPBF16)
        in_maps.append(dict(xi=xi, w1=w1i, w2=w2blk, b2=b2q, wl=wlrep))
    return in_maps


def kernel(**inputs):
    in_maps = _make_in_maps(inputs)

    if "nc" not in _CACHE:
        _CACHE["nc"] = _build_graph()
    nc = _CACHE["nc"]

    results = bass_utils.run_bass_kernel_spmd(
        nc, in_maps, core_ids=list(range(NCORES))
    )
    total = np.float64(0.0)
    for r in results.results:
        total += np.asarray(r["out"], np.float64).sum()
    mse = total / (NKP * B * 128 * (COUT * COUT) ** 2)
    return np.asarray(mse, np.float32)


if __name__ == "__main__":
    rng = np.random.default_rng(0)
    ins = dict(
        images_ground=rng.standard_normal((B, 3, H, H)).astype(np.float32),
        images_satellite=rng.standard_normal((B, 3, H, H)).astype(np.float32),
        keypoints_ground=(0.2 + 0.6 * rng.random((B, K, 2))).astype(np.float32),
        keypoints_satellite=(0.2 + 0.6 * rng.random((B, K, 2))).astype(np.float32),
        w1=(rng.standard_normal((32, 3, 3, 3)) * 0.1).astype(np.float32),
        b1=np.zeros(32, np.float32),
        w2=(rng.standard_normal((64, 32, 3, 3)) * 0.05).astype(np.float32),
        b2=np.zeros(64, np.float32),
        wl=(rng.standard_normal((128, 64)) * 0.1).astype(np.float32),
        bl=np.zeros(128, np.float32),
        num_samples=K,
    )
    print("kernel out:", kernel(**ins))


# revision 44
# speedup vs baseline: 1.0430x; 1.0076x over previous
"""Trainium2 Bass kernel for nn_AppearanceLoss (keypoint patch CNN MSE).

Host: crops 33x33 patches at keypoint locations, builds full im2col
(27 rows = 3c x 3dy x 3dx per patch) so conv1 is a single-shot matmul,
shards 256 keypoints across 8 NeuronCores.

Device (v2): group-PAIR structure (2 groups = 16 patches per iteration,
ground/satellite pairs interleaved so the MSE endgame chunks through the
kernel). conv1 = 64x64 PE-tiling, 4 concurrent tiles; conv2 =
offset-accumulated K=64 block-diag matmuls, one weight load per
(offset, row-half) serving both groups of the pair (a post-build pass
deletes redundant LDWEIGHTS). conv2 eviction = relu+bias to bf16
scratch on ACT/DVE + segmented GAP reduce on GpSimd. Linear on feature
diffs computed in 4 chunks during steady state; output DMA split in two.
Host sums 8 per-core partials into the scalar MSE.
"""

import sys

sys.path.insert(0, "/opt/trn_rl_repo")

from contextlib import ExitStack

import ml_dtypes
import numpy as np

import concourse.bass as bass  # noqa: F401
import concourse.tile as tile
from concourse import bacc, bass_utils, mybir

SIGMA = 16
PATCH = 33  # 2*SIGMA+1
HOUT = 31  # conv1 valid output: 33-3+1
COUT = 15  # conv2 stride-2 valid output: (31-3)//2+1
B, K, H = 4, 64, 256
NCORES = 8
NKP = B * K  # 256 keypoints total
KPC = NKP // NCORES  # 32 keypoints per core
NPATCH = KPC * B  # 128 patches per core per set
NG = 32  # groups of 8 patches (16 ground + 16 sat)
NGP = NG // 2  # 16 group-pairs
KIM = 55  # conv1 im2col rows per pair: 2*27 + ones
BF16 = mybir.dt.bfloat16
F32 = mybir.dt.float32
NPBF16 = ml_dtypes.bfloat16

_CACHE: dict = {}


def _dedupe_ldweights(nc):
    """Remove InstLdweights that reload weights already resident in the
    same PE tile position (identical access pattern, no intervening
    overlapping load). Waits on a removed load move to the next
    instruction (its matmul); loads with updates are kept."""
    removed = 0
    for blk in nc.main_func.blocks:
        referenced = set()
        for inst in blk.instructions:
            try:
                for name, _ in inst.dependency_edges():
                    referenced.add(name)
            except Exception:
                pass
        live = {}  # tile_position -> (signature, rect)

        def overlap(a, b):
            return a[0] < b[1] and b[0] < a[1] and a[2] < b[3] and b[2] < a[3]

        insts = blk.instructions
        keep = []
        for idx, inst in enumerate(insts):
            tname = type(inst).__name__
            if tname != "InstLdweights":
                keep.append(inst)
                continue
            tp = inst.tile_position
            ts = inst.tile_size
            if tp is None or ts is None:
                live.clear()
                keep.append(inst)
                continue
            rect = (tp[0], tp[0] + ts[0], tp[1], tp[1] + ts[1])
            sig = (
                tuple(tp),
                tuple(ts),
                inst.perf_mode,
                inst.is_transpose,
                str(inst.ins[0]),
            )
            si = inst.sync_info
            has_update = si is not None and len(si.on_update) > 0
            prev = live.get(tuple(tp))
            if (
                prev is not None
                and prev[0] == sig
                and not has_update
                and inst.name not in referenced
            ):
                # redundant: same weights already loaded at this position
                waits = list(si.on_wait) if si is not None else []
                if waits:
                    # move waits onto the next instruction (its matmul)
                    nxt = insts[idx + 1]
                    nsi = nxt.sync_info
                    if nsi is None:
                        nxt.sync_info = mybir.SyncInfo(
                            on_wait=waits, on_update=[]
                        )
                    else:
                        nsi.on_wait = list(nsi.on_wait) + waits
                removed += 1
                continue
            # invalidate everything this load overlaps, then record it
            for k in [k for k, v in live.items() if overlap(v[1], rect)]:
                del live[k]
            live[tuple(tp)] = (sig, rect)
            keep.append(inst)
        if removed:
            blk.instructions[:] = keep
    return removed


def _strip_matmul_sem_incs(nc):
    """Every matmul carries a +1 update on the PE progress semaphore;
    each update is a serialized EVT_SEM register write (~26ns) that
    inflates the back-to-back matmul round time. Only the increments
    whose cumulative count is referenced by some wait threshold are
    needed. Keep those (and the last), strip the rest, and remap all
    wait thresholds to the kept-increment numbering."""
    import collections

    mm_types = ("InstMatmult",)
    # gather per-sem: ordered updater list (must be all matmuls), waits
    upd_by_sem = collections.defaultdict(list)  # sem_id -> [inst,...]
    waits_by_sem = collections.defaultdict(list)  # sem_id -> [(inst, wi)]
    bad_sems = set()
    all_insts = []
    for blk in nc.main_func.blocks:
        all_insts.extend(blk.instructions)
    for inst in all_insts:
        si = inst.sync_info
        if si is None:
            continue
        for u in si.on_update:
            if u.sync_type != "semaphore":
                continue
            sid = int(u.id)
            if u.update_mode == "sem-inc" and u.update_reg is None:
                if type(inst).__name__ in mm_types and u.update_value == 1:
                    upd_by_sem[sid].append(inst)
                else:
                    bad_sems.add(sid)
            elif u.update_mode != "sem-set":
                # teardown resets (sem-set) are fine; anything else isn't
                bad_sems.add(sid)
        for wi, w in enumerate(si.on_wait):
            if w.sync_type != "semaphore":
                continue
            sid = int(w.id)
            waits_by_sem[sid].append((inst, wi))
            if w.wait_mode != "sem-ge-imm" or w.wait_reg is not None:
                bad_sems.add(sid)

    stripped = 0
    for sid, updaters in upd_by_sem.items():
        if sid in bad_sems or len(updaters) < 8:
            continue
        referenced = set()
        ok = True
        for inst, wi in waits_by_sem.get(sid, []):
            v = inst.sync_info.on_wait[wi].wait_value
            if v is None or v < 0 or v > len(updaters):
                ok = False
                break
            if v >= 1:
                referenced.add(int(v))
        if not ok:
            continue
        referenced.add(len(updaters))  # keep the final increment
        # map old cumulative count -> new cumulative count
        keep = sorted(referenced)
        newcount = {}
        kept_so_far = 0
        ki = 0
        for oldc in range(1, len(updaters) + 1):
            if ki < len(keep) and keep[ki] == oldc:
                kept_so_far += 1
                ki += 1
            newcount[oldc] = kept_so_far
        # strip updates from non-kept matmuls
        keepset = referenced
        for idx, inst in enumerate(updaters):
            oldc = idx + 1
            if oldc in keepset:
                continue
            si = inst.sync_info
            si.on_update = [
                u
                for u in si.on_update
                if not (u.sync_type == "semaphore" and int(u.id) == sid)
            ]
            stripped += 1
        # remap wait thresholds
        for inst, wi in waits_by_sem.get(sid, []):
            w = inst.sync_info.on_wait[wi]
            if int(w.wait_value) >= 1:
                w.wait_value = newcount[int(w.wait_value)]
    return stripped


def _build_graph():
    nc = bacc.Bacc(
        "TRN2",
        target_bir_lowering=False,
        debug=False,
        enable_asserts=False,
        num_devices=NCORES,
    )
    # conv1 im2col input per group-pair GP: partition 64R + 27a + k holds
    # im2col row k (k = 9c+3dy+dx) of patch (GP,R,g,cg,a); partition
    # 64R+54 = 1.0 (bias row); free dims [g in-pair group, cg pair-sel,
    # 31 y, 31 x].
    xi_d = nc.dram_tensor(
        "xi", [NGP, 2, KIM, 2, 2, HOUT, HOUT], BF16, kind="ExternalInput"
    ).ap()
    w1_d = nc.dram_tensor("w1", [128, 64], BF16, kind="ExternalInput").ap()
    w2_d = nc.dram_tensor("w2", [128, 9 * 128], BF16, kind="ExternalInput").ap()
    b2_d = nc.dram_tensor("b2", [128, 1], F32, kind="ExternalInput").ap()
    wl_d = nc.dram_tensor("wl", [128, 128], BF16, kind="ExternalInput").ap()
    out_d = nc.dram_tensor("out", [128, 8], F32, kind="ExternalOutput").ap()

    RELU = mybir.ActivationFunctionType.Relu
    SQUARE = mybir.ActivationFunctionType.Square
    ADD = mybir.AluOpType.add
    MAX = mybir.AluOpType.max
    SUB = mybir.AluOpType.subtract

    # emission order of group-pairs: ground gp, sat gp interleaved.
    # group-pair gp covers groups (2gp, 2gp+1); ground gps 0-7, sat 8-15.
    GP_ORDER = []
    for i in range(8):
        GP_ORDER.append(i)
        GP_ORDER.append(8 + i)

    with ExitStack() as ctx:
        tc = ctx.enter_context(tile.TileContext(nc))
        const = ctx.enter_context(tc.tile_pool(name="const", bufs=1))
        xpool = ctx.enter_context(tc.tile_pool(name="x", bufs=4))
        hpool = ctx.enter_context(tc.tile_pool(name="h", bufs=3))
        gpool = ctx.enter_context(tc.tile_pool(name="g", bufs=1))
        spool = ctx.enter_context(tc.tile_pool(name="scr", bufs=4))
        pp1 = ctx.enter_context(tc.tile_pool(name="pp1", bufs=4, space="PSUM"))
        pp2 = ctx.enter_context(tc.tile_pool(name="pp2", bufs=4, space="PSUM"))

        xi_tiles: dict = {}

        def issue_dma(i):
            # split per (R, g): 4 transfers per pair, R0 pieces on the
            # sync queue and R1 on gpsimd so both DMA rings pull
            gp = GP_ORDER[i]
            xt = xpool.tile(
                [128, 2, 2, HOUT, HOUT], BF16, tag="xi", name=f"xi_{gp}"
            )
            for R, e in ((0, nc.sync), (1, nc.gpsimd)):
                for g in range(2):
                    e.dma_start(
                        xt[64 * R : 64 * R + KIM, g], xi_d[gp, R, :, g]
                    )
            xi_tiles[gp] = xt

        # --- consts + first group-pairs, spread across the three
        # DMA-capable queues so the first conv1 data lands asap:
        # sync: w1 then gp0's R0; gpsimd: gp0's R1 then gp8;
        # scalar: w2 then gp1, then b2/wl ---
        # scalar queue gets ONLY the three tiny const DMAs -- any big xi
        # issue there jams the eviction FIFO behind DMA-sem recycling.
        # xi gp0 is issued ALONE first so it monopolizes the DMA fabric
        # (the engines round-robin; co-issued pairs all finish late).
        w1_t = const.tile([128, 64], BF16)
        nc.sync.dma_start(w1_t[:], w1_d)
        issue_dma(0)
        w2_t = const.tile([128, 9 * 128], BF16)
        nc.scalar.dma_start(w2_t[:], w2_d)
        b2_t = const.tile([128, 1], F32)
        nc.scalar.dma_start(b2_t[:], b2_d)
        wl_t = const.tile([128, 128], BF16)
        nc.scalar.dma_start(wl_t[:], wl_d)
        issue_dma(1)
        issue_dma(2)

        # gap col layout: for group G (= 2gp+g), jj, q: col 4G+2jj+q;
        # partition 64a+m = patch (G, q, jj?, a)... (cols are summed
        # symmetrically on host, only ground<->sat pairing must match)
        gap = gpool.tile([128, NG * 4], F32)
        res = gpool.tile([128, 8], F32)
        wres = gpool.tile([128, 1], F32)  # warmup sink, never DMA'd

        # PE warm-up burst: keeps the PE busy from the end of the
        # framework prologue until the first conv1 matmul so the HAM
        # clock gate reaches 8/8 as early as possible
        junk = const.tile([128, 320], BF16, name="junk")
        nc.vector.memset(junk[:], 0.5)
        wps = pp1.tile([128, 320], F32, tag="ps1", name="warm_ps")
        for i in range(5):
            nc.tensor.matmul(
                wps[:],
                junk[:, 0:128],
                junk[:],
                start=(i == 0),
                stop=(i == 4),
            )
        wscr = spool.tile([128, 320], F32, tag="wscr")
        nc.scalar.activation(wscr[:], wps[:], SQUARE, accum_out=wres[:, 0:1])

        # greedy ACT/DVE load balancing on estimated busy-ns
        eng_ns = {"act": 0.0, "dve": 0.0}

        def evict_relu(dst, src):
            # conv1 eviction: relu, f32 PSUM -> bf16 SBUF
            if eng_ns["act"] + 630 <= eng_ns["dve"] + 660:
                eng_ns["act"] += 630
                nc.scalar.activation(dst, src, RELU)
            else:
                eng_ns["dve"] += 660
                nc.vector.tensor_scalar_max(dst, src, 0.0)

        def emit_conv1_g(gp, xt, h1, g):
            # conv1 for one group of a pair: 2 y-half waves of 4
            # concurrent 64x64 PE tiles (2R x 2cg); per-wave 2 PSUM
            # tiles so the 4-slot pool keeps 2 waves of slack, and the
            # g=0 block only needs half the pair's xi data (head ramp)
            for y0, nr in ((0, 16), (16, 15)):
                pss = {}
                for R in range(2):
                    ps = pp1.tile(
                        [128, nr, HOUT], F32, tag="ps1", name=f"c1_{g}{R}"
                    )
                    for cg in range(2):
                        nc.tensor.matmul(
                            ps[64 * cg : 64 * cg + 64, :, :],
                            w1_t[64 * R : 64 * R + KIM, :],
                            xt[
                                64 * R : 64 * R + KIM,
                                g,
                                cg,
                                y0 : y0 + nr,
                                :,
                            ],
                            start=True,
                            stop=True,
                            tile_position=(64 * R, 64 * cg),
                        )
                    pss[R] = ps
                for R in range(2):
                    evict_relu(
                        h1[:, g, R, y0 : y0 + nr, :], pss[R][:, :, :]
                    )

        c2_turn = [0]

        def emit_conv2_phase(gp, h1, g):
            # conv2 for one group of a pair: all 9 offsets, then evict.
            # Using only 2 of the 4 pp2 slots per phase leaves the pool
            # double-buffered across phases, so o=0 never stalls on
            # the previous phase's evictions.
            ps2s = {
                jj: pp2.tile(
                    [128, 2, COUT * COUT], F32, tag="ps2", name=f"ps2_{g}{jj}"
                )
                for jj in range(2)
            }
            for o in range(9):
                dy, dx = o // 3, o % 3
                for jj in range(2):
                    p0 = 64 * jj
                    nc.tensor.matmul(
                        ps2s[jj][:],
                        w2_t[p0 : p0 + 64, 128 * o : 128 * o + 128],
                        h1[
                            p0 : p0 + 64,
                            g,
                            :,
                            dy : dy + 29 : 2,
                            dx : dx + 29 : 2,
                        ],
                        start=(o == 0),
                        stop=(o == 8),
                        tile_position=(p0, 0),
                    )
            # eviction: relu(x + b2) then GAP sum into 2 gap columns.
            # NOTE: DVE accum_out is broken on TRN2 hardware (and
            # clobbers op1) -- only ACT may use accum_out. Plans:
            #  a) ACT relu->bf16 scratch + DVE segmented reduce
            #  b) DVE relu->bf16 scratch + DVE segmented reduce
            #  c) ACT in-place relu+bias+accum per q (no scratch)
            G = 2 * gp + g
            for jj in range(2):
                src = ps2s[jj]
                col = 4 * G + 2 * jj
                costs = {
                    "a": max(eng_ns["act"] + 630, eng_ns["dve"] + 613),
                    "b": eng_ns["dve"] + 1258,
                    "c": eng_ns["act"] + 1834,
                }
                plan = min(costs, key=costs.get)
                if plan == "c":
                    eng_ns["act"] += 1374
                    for q in range(2):
                        nc.scalar.activation(
                            src[:, q, :],
                            src[:, q, :],
                            RELU,
                            bias=b2_t[:],
                            accum_out=gap[:, col + q : col + q + 1],
                        )
                    continue
                scr = spool.tile(
                    [128, 2, COUT * COUT], BF16, tag="scr2", name="scr2"
                )
                if plan == "a":
                    eng_ns["act"] += 630
                    nc.scalar.activation(scr[:], src[:], RELU, bias=b2_t[:])
                else:
                    eng_ns["dve"] += 645
                    nc.vector.tensor_scalar(
                        scr[:], src[:], b2_t[:], 0.0, op0=ADD, op1=MAX
                    )
                eng_ns["dve"] += 613
                nc.vector.tensor_reduce(
                    gap[:, col : col + 2],
                    scr[:],
                    axis=mybir.AxisListType.X,
                    op=ADD,
                )

        def emit_chunk_diff(c):
            # feature diffs for 16 ground cols [16c, 16c+16) paired with
            # sat cols [64+16c, 80+16c), on GpSimd (SBUF-only engine)
            c0 = 16 * c
            dg = spool.tile([128, 16], F32, tag="dg", name=f"dg_{c}")
            dgb = spool.tile([128, 16], BF16, tag="dgb", name=f"dgb_{c}")
            nc.gpsimd.tensor_tensor(
                dg[:], gap[:, c0 : c0 + 16], gap[:, 64 + c0 : 80 + c0], op=SUB
            )
            nc.gpsimd.tensor_copy(dgb[:], dg[:])
            return dgb

        def emit_chunk_mm(c, dgb):
            # linear + square for a finished chunk (deferred so the PE
            # queue never waits on the GpSimd diff chain).
            for jj in range(2):
                p0 = 64 * jj
                ps3 = pp2.tile([128, 16], F32, tag="ps2", name=f"ps3_{c}{jj}")
                nc.tensor.matmul(
                    ps3[:],
                    wl_t[p0 : p0 + 64, :],
                    dgb[p0 : p0 + 64, :],
                    start=True,
                    stop=True,
                    tile_position=(p0, 0),
                )
                scr3 = spool.tile(
                    [128, 16], F32, tag="scr3", name=f"scr3_{c}{jj}"
                )
                nc.scalar.activation(
                    scr3[:], ps3[:], SQUARE, accum_out=res[:, 2 * c + jj : 2 * c + jj + 1]
                )
            # 2 unused pad allocations keep the 4-slot pp2 rotation
            # parity; they land on the in-flight g0 slots but are never
            # written or read, so they cannot stall anything
            for _pad in range(2):
                pp2.tile([128, 16], F32, tag="ps2", name=f"pad_{c}{_pad}")

        # software-pipelined emission: conv1 y-waves of pair i interleave
        # with the two conv2 group-phases of pair i-2; endgame chunk
        # after every 2nd sat group-pair
        pending = []
        done_sat = [0]
        chunk_q = []  # (c, dgb) whose PE part is deferred

        def flush_chunks():
            while chunk_q:
                c, dgb = chunk_q.pop(0)
                emit_chunk_mm(c, dgb)
                if c == 1:
                    nc.sync.dma_start(out_d[:, 0:4], res[:, 0:4])

        def after_conv2(gp2):
            if gp2 >= 8:
                done_sat[0] += 1
                if done_sat[0] % 2 == 0:
                    c = done_sat[0] // 2 - 1
                    chunk_q.append((c, emit_chunk_diff(c)))

        # per-iteration conv2 phase schedule: the pipeline fills with
        # two half-iterations (phase g0 of gp0 at i=1, g1 at i=2) so
        # the depth-2 restore bubble lands in the DMA-limited ramp
        phase_q = []  # (gp, h1, g) conv2 phases awaiting emission

        def pop_phase():
            if phase_q:
                gp2, h2, g2 = phase_q.pop(0)
                emit_conv2_phase(gp2, h2, g2)
                if g2 == 1:
                    after_conv2(gp2)

        for i in range(NGP):
            gp = GP_ORDER[i]
            xt = xi_tiles.pop(gp)
            h1 = hpool.tile(
                [128, 2, 2, HOUT, HOUT], BF16, tag="h1", name=f"h1_{gp}"
            )
            emit_conv1_g(gp, xt, h1, 0)
            if i + 3 < NGP:
                issue_dma(i + 3)
            if i >= 1:
                pop_phase()
            flush_chunks()
            emit_conv1_g(gp, xt, h1, 1)
            phase_q.append((gp, h1, 0))
            phase_q.append((gp, h1, 1))
            if i >= 3:
                pop_phase()
        while phase_q:
            pop_phase()
            flush_chunks()
        nc.sync.dma_start(out_d[:, 4:8], res[:, 4:8])

    ndup = _dedupe_ldweights(nc)
    nsem = _strip_matmul_sem_incs(nc)
    print(
        f"[kernel] deduped {ndup} LDWEIGHTS, stripped {nsem} sem incs",
        file=sys.stderr,
    )
    nc.compile()
    return nc


def _prep_weights(w1, b1, w2, b2, wl):
    # conv1 im2col weights: [64R + 27a + (9c+3dy+3?dx), 32a+m]
    w1i = np.zeros((128, 64), np.float32)
    for a in range(2):
        for c in range(3):
            for dy in range(3):
                for dx in range(3):
                    w1i[27 * a + 9 * c + 3 * dy + dx, 32 * a : 32 * a + 32] = w1[
                        :, c, dy, dx
                    ]
        w1i[54, 32 * a : 32 * a + 32] = b1
    w1i[64:119] = w1i[0:55]

    w2blk = np.zeros((128, 9, 128), np.float32)
    for jj in range(2):
        for j in range(2):
            for c in range(32):
                for o in range(9):
                    dy, dx = o // 3, o % 3
                    w2blk[64 * jj + 32 * j + c, o, 64 * j : 64 * j + 64] = w2[
                        :, c, dy, dx
                    ]
    b2q = np.tile(b2, 2)[:, None].astype(np.float32)  # unscaled
    wlrep = np.zeros((128, 128), np.float32)
    wlrep[0:64] = wl.T
    wlrep[64:128] = wl.T
    return (
        w1i.astype(NPBF16),
        w2blk.reshape(128, 9 * 128).astype(NPBF16),
        np.ascontiguousarray(b2q),
        wlrep.astype(NPBF16),
    )


def _crop_all(images, kps):
    # images [B,3,H,W] f32; kps [NKP,2] normalized -> patches [NKP,B,3,P,P]
    hw = images.shape[-1]
    px = kps.astype(np.float32) * np.float32(hw)
    starts = np.clip(np.floor(px).astype(np.int32) - SIGMA, 0, hw - PATCH)
    out = np.empty((kps.shape[0], images.shape[0], 3, PATCH, PATCH), np.float32)
    for n in range(kps.shape[0]):
        x, y = int(starts[n, 0]), int(starts[n, 1])
        out[n] = images[:, :, y : y + PATCH, x : x + PATCH]
    return out


def _im2col_groups(pat):
    # pat [128,3,33,33] (one set for one core) -> [8, 2, 55, 2, 2, 31, 31]
    # (gp, R, im2col row (27a+9c+3dy+dx | 54=ones), g in-pair, cg, y, x)
    sw = np.lib.stride_tricks.sliding_window_view(pat, (HOUT, HOUT), axis=(2, 3))
    # sw[n, c, dy, dx, y, x] = pat[n, c, dy+y, dx+x]
    sw = sw.reshape(8, 2, 2, 2, 2, 27, HOUT, HOUT)  # (gp, g, R, cg, a, k, y, x)
    out = np.empty((8, 2, KIM, 2, 2, HOUT, HOUT), np.float32)
    # target row = 27a + k; dims (gp, R, a, k, g, cg, y, x)
    out[:, :, :54] = sw.transpose(0, 2, 4, 5, 1, 3, 6, 7).reshape(
        8, 2, 54, 2, 2, HOUT, HOUT
    )
    out[:, :, 54] = 1.0
    return out


def _make_in_maps(np_inputs):
    images_ground = np.asarray(np_inputs["images_ground"], np.float32)
    images_satellite = np.asarray(np_inputs["images_satellite"], np.float32)
    kg = np.asarray(np_inputs["keypoints_ground"], np.float32).reshape(-1, 2)
    ks = np.asarray(np_inputs["keypoints_satellite"], np.float32).reshape(-1, 2)
    w1 = np.asarray(np_inputs["w1"], np.float32)
    b1 = np.asarray(np_inputs["b1"], np.float32)
    w2 = np.asarray(np_inputs["w2"], np.float32)
    b2 = np.asarray(np_inputs["b2"], np.float32)
    wl = np.asarray(np_inputs["wl"], np.float32)

    pg = _crop_all(images_ground, kg)  # [256,4,3,33,33]
    ps = _crop_all(images_satellite, ks)
    w1i, w2blk, b2q, wlrep = _prep_weights(w1, b1, w2, b2, wl)

    in_maps = []
    for i in range(NCORES):
        sl = slice(i * KPC, (i + 1) * KPC)
        patg = pg[sl].reshape(NPATCH, 3, PATCH, PATCH)
        pats = ps[sl].reshape(NPATCH, 3, PATCH, PATCH)
        xi = np.concatenate(
            [_im2col_groups(patg), _im2col_groups(pats)], axis=0
        ).astype(NPBF16)
        in_maps.append(dict(xi=xi, w1=w1i, w2=w2blk, b2=b2q, wl=wlrep))
    return in_maps


def kernel(**inputs):
    in_maps = _make_in_maps(inputs)

    if "nc" not in _CACHE:
        _CACHE["nc"] = _build_graph()
    nc = _CACHE["nc"]

    results = bass_utils.run_bass_kernel_spmd(
        nc, in_maps, core_ids=list(range(NCORES))
    )
    total = np.float64(0.0)
    for r in results.results:
        total += np.asarray(r["out"], np.float64).sum()
    mse = total / (NKP * B * 128 * (COUT * COUT) ** 2)
    return np.asarray(mse, np.float32)


if __name__ == "__main__":
    rng = np.random.default_rng(0)
    ins = dict(
        images_ground=rng.standard_normal((B, 3, H, H)).astype(np.float32),
        images_satellite=rng.standard_normal((B, 3, H, H)).astype(np.float32),
        keypoints_ground=(0.2 + 0.6 * rng.random((B, K, 2))).astype(np.float32),
        keypoints_satellite=(0.2 + 0.6 * rng.random((B, K, 2))).astype(np.float32),
        w1=(rng.standard_normal((32, 3, 3, 3)) * 0.1).astype(np.float32),
        b1=np.zeros(32, np.float32),
        w2=(rng.standard_normal((64, 32, 3, 3)) * 0.05).astype(np.float32),
        b2=np.zeros(64, np.float32),
        wl=(rng.standard_normal((128, 64)) * 0.1).astype(np.float32),
        bl=np.zeros(128, np.float32),
        num_samples=K,
    )
    print("kernel out:", kernel(**ins))


# revision 45
# speedup vs baseline: 1.0671x; 1.0231x over previous
"""Trainium2 Bass kernel for nn_AppearanceLoss (keypoint patch CNN MSE).

Host: crops 33x33 patches at keypoint locations, builds full im2col
(27 rows = 3c x 3dy x 3dx per patch) so conv1 is a single-shot matmul,
shards 256 keypoints across 8 NeuronCores.

Device (v2): group-PAIR structure (2 groups = 16 patches per iteration,
ground/satellite pairs interleaved so the MSE endgame chunks through the
kernel). conv1 = 64x64 PE-tiling, 4 concurrent tiles; conv2 =
offset-accumulated K=64 block-diag matmuls, one weight load per
(offset, row-half) serving both groups of the pair (a post-build pass
deletes redundant LDWEIGHTS). conv2 eviction = relu+bias to bf16
scratch on ACT/DVE + segmented GAP reduce on GpSimd. Linear on feature
diffs computed in 4 chunks during steady state; output DMA split in two.
Host sums 8 per-core partials into the scalar MSE.
"""

import sys

sys.path.insert(0, "/opt/trn_rl_repo")

from contextlib import ExitStack

import ml_dtypes
import numpy as np

import concourse.bass as bass  # noqa: F401
import concourse.tile as tile
from concourse import bacc, bass_utils, mybir

SIGMA = 16
PATCH = 33  # 2*SIGMA+1
HOUT = 31  # conv1 valid output: 33-3+1
COUT = 15  # conv2 stride-2 valid output: (31-3)//2+1
B, K, H = 4, 64, 256
NCORES = 8
NKP = B * K  # 256 keypoints total
KPC = NKP // NCORES  # 32 keypoints per core
NPATCH = KPC * B  # 128 patches per core per set
NG = 32  # groups of 8 patches (16 ground + 16 sat)
NGP = NG // 2  # 16 group-pairs
KIM = 55  # conv1 im2col rows per pair: 2*27 + ones
BF16 = mybir.dt.bfloat16
F32 = mybir.dt.float32
NPBF16 = ml_dtypes.bfloat16

_CACHE: dict = {}


def _dedupe_ldweights(nc):
    """Remove InstLdweights that reload weights already resident in the
    same PE tile position (identical access pattern, no intervening
    overlapping load). Waits on a removed load move to the next
    instruction (its matmul); loads with updates are kept."""
    removed = 0
    for blk in nc.main_func.blocks:
        referenced = set()
        for inst in blk.instructions:
            try:
                for name, _ in inst.dependency_edges():
                    referenced.add(name)
            except Exception:
                pass
        live = {}  # tile_position -> (signature, rect)

        def overlap(a, b):
            return a[0] < b[1] and b[0] < a[1] and a[2] < b[3] and b[2] < a[3]

        insts = blk.instructions
        keep = []
        for idx, inst in enumerate(insts):
            tname = type(inst).__name__
            if tname != "InstLdweights":
                keep.append(inst)
                continue
            tp = inst.tile_position
            ts = inst.tile_size
            if tp is None or ts is None:
                live.clear()
                keep.append(inst)
                continue
            rect = (tp[0], tp[0] + ts[0], tp[1], tp[1] + ts[1])
            sig = (
                tuple(tp),
                tuple(ts),
                inst.perf_mode,
                inst.is_transpose,
                str(inst.ins[0]),
            )
            si = inst.sync_info
            has_update = si is not None and len(si.on_update) > 0
            prev = live.get(tuple(tp))
            if (
                prev is not None
                and prev[0] == sig
                and not has_update
                and inst.name not in referenced
            ):
                # redundant: same weights already loaded at this position
                waits = list(si.on_wait) if si is not None else []
                if waits:
                    # move waits onto the next instruction (its matmul)
                    nxt = insts[idx + 1]
                    nsi = nxt.sync_info
                    if nsi is None:
                        nxt.sync_info = mybir.SyncInfo(
                            on_wait=waits, on_update=[]
                        )
                    else:
                        nsi.on_wait = list(nsi.on_wait) + waits
                removed += 1
                continue
            # invalidate everything this load overlaps, then record it
            for k in [k for k, v in live.items() if overlap(v[1], rect)]:
                del live[k]
            live[tuple(tp)] = (sig, rect)
            keep.append(inst)
        if removed:
            blk.instructions[:] = keep
    return removed


def _strip_matmul_sem_incs(nc):
    """Every matmul carries a +1 update on the PE progress semaphore;
    each update is a serialized EVT_SEM register write (~26ns) that
    inflates the back-to-back matmul round time. Only the increments
    whose cumulative count is referenced by some wait threshold are
    needed. Keep those (and the last), strip the rest, and remap all
    wait thresholds to the kept-increment numbering."""
    import collections

    mm_types = ("InstMatmult",)
    # gather per-sem: ordered updater list (must be all matmuls), waits
    upd_by_sem = collections.defaultdict(list)  # sem_id -> [inst,...]
    waits_by_sem = collections.defaultdict(list)  # sem_id -> [(inst, wi)]
    bad_sems = set()
    all_insts = []
    for blk in nc.main_func.blocks:
        all_insts.extend(blk.instructions)
    for inst in all_insts:
        si = inst.sync_info
        if si is None:
            continue
        for u in si.on_update:
            if u.sync_type != "semaphore":
                continue
            sid = int(u.id)
            if u.update_mode == "sem-inc" and u.update_reg is None:
                if type(inst).__name__ in mm_types and u.update_value == 1:
                    upd_by_sem[sid].append(inst)
                else:
                    bad_sems.add(sid)
            elif u.update_mode != "sem-set":
                # teardown resets (sem-set) are fine; anything else isn't
                bad_sems.add(sid)
        for wi, w in enumerate(si.on_wait):
            if w.sync_type != "semaphore":
                continue
            sid = int(w.id)
            waits_by_sem[sid].append((inst, wi))
            if w.wait_mode != "sem-ge-imm" or w.wait_reg is not None:
                bad_sems.add(sid)

    stripped = 0
    for sid, updaters in upd_by_sem.items():
        if sid in bad_sems or len(updaters) < 8:
            continue
        referenced = set()
        ok = True
        for inst, wi in waits_by_sem.get(sid, []):
            v = inst.sync_info.on_wait[wi].wait_value
            if v is None or v < 0 or v > len(updaters):
                ok = False
                break
            if v >= 1:
                referenced.add(int(v))
        if not ok:
            continue
        referenced.add(len(updaters))  # keep the final increment
        # map old cumulative count -> new cumulative count
        keep = sorted(referenced)
        newcount = {}
        kept_so_far = 0
        ki = 0
        for oldc in range(1, len(updaters) + 1):
            if ki < len(keep) and keep[ki] == oldc:
                kept_so_far += 1
                ki += 1
            newcount[oldc] = kept_so_far
        # strip updates from non-kept matmuls
        keepset = referenced
        for idx, inst in enumerate(updaters):
            oldc = idx + 1
            if oldc in keepset:
                continue
            si = inst.sync_info
            si.on_update = [
                u
                for u in si.on_update
                if not (u.sync_type == "semaphore" and int(u.id) == sid)
            ]
            stripped += 1
        # remap wait thresholds
        for inst, wi in waits_by_sem.get(sid, []):
            w = inst.sync_info.on_wait[wi]
            if int(w.wait_value) >= 1:
                w.wait_value = newcount[int(w.wait_value)]
    return stripped


def _build_graph():
    nc = bacc.Bacc(
        "TRN2",
        target_bir_lowering=False,
        debug=False,
        enable_asserts=False,
        num_devices=NCORES,
    )
    # conv1 im2col input per group-pair GP: partition 64R + 27a + k holds
    # im2col row k (k = 9c+3dy+dx) of patch (GP,R,g,cg,a); partition
    # 64R+54 = 1.0 (bias row); free dims [g in-pair group, cg pair-sel,
    # 31 y, 31 x].
    xi_d = nc.dram_tensor(
        "xi", [NGP, 2, KIM, 2, 2, HOUT, HOUT], BF16, kind="ExternalInput"
    ).ap()
    w1_d = nc.dram_tensor("w1", [128, 64], BF16, kind="ExternalInput").ap()
    w2_d = nc.dram_tensor("w2", [128, 9 * 128], BF16, kind="ExternalInput").ap()
    b2_d = nc.dram_tensor("b2", [128, 1], F32, kind="ExternalInput").ap()
    wl_d = nc.dram_tensor("wl", [128, 128], BF16, kind="ExternalInput").ap()
    out_d = nc.dram_tensor("out", [128, 8], F32, kind="ExternalOutput").ap()

    RELU = mybir.ActivationFunctionType.Relu
    SQUARE = mybir.ActivationFunctionType.Square
    ADD = mybir.AluOpType.add
    MAX = mybir.AluOpType.max
    SUB = mybir.AluOpType.subtract

    # emission order of group-pairs: ground gp, sat gp interleaved.
    # group-pair gp covers groups (2gp, 2gp+1); ground gps 0-7, sat 8-15.
    GP_ORDER = []
    for i in range(8):
        GP_ORDER.append(i)
        GP_ORDER.append(8 + i)

    with ExitStack() as ctx:
        tc = ctx.enter_context(tile.TileContext(nc))
        const = ctx.enter_context(tc.tile_pool(name="const", bufs=1))
        xpool = ctx.enter_context(tc.tile_pool(name="x", bufs=4))
        hpool = ctx.enter_context(tc.tile_pool(name="h", bufs=3))
        gpool = ctx.enter_context(tc.tile_pool(name="g", bufs=1))
        spool = ctx.enter_context(tc.tile_pool(name="scr", bufs=4))
        pp1 = ctx.enter_context(tc.tile_pool(name="pp1", bufs=4, space="PSUM"))
        pp2 = ctx.enter_context(tc.tile_pool(name="pp2", bufs=4, space="PSUM"))

        xi_tiles: dict = {}

        def issue_dma(i):
            # split per (R, g): 4 transfers per pair, R0 pieces on the
            # sync queue and R1 on gpsimd so both DMA rings pull
            gp = GP_ORDER[i]
            xt = xpool.tile(
                [128, 2, 2, HOUT, HOUT], BF16, tag="xi", name=f"xi_{gp}"
            )
            for R, e in ((0, nc.sync), (1, nc.gpsimd)):
                for g in range(2):
                    e.dma_start(
                        xt[64 * R : 64 * R + KIM, g], xi_d[gp, R, :, g]
                    )
            xi_tiles[gp] = xt

        # --- consts + first group-pairs, spread across the three
        # DMA-capable queues so the first conv1 data lands asap:
        # sync: w1 then gp0's R0; gpsimd: gp0's R1 then gp8;
        # scalar: w2 then gp1, then b2/wl ---
        # scalar queue gets ONLY the three tiny const DMAs -- any big xi
        # issue there jams the eviction FIFO behind DMA-sem recycling.
        # xi gp0 is issued ALONE first so it monopolizes the DMA fabric
        # (the engines round-robin; co-issued pairs all finish late).
        w1_t = const.tile([128, 64], BF16)
        nc.sync.dma_start(w1_t[:], w1_d)
        issue_dma(0)
        w2_t = const.tile([128, 9 * 128], BF16)
        nc.scalar.dma_start(w2_t[:], w2_d)
        b2_t = const.tile([128, 1], F32)
        nc.scalar.dma_start(b2_t[:], b2_d)
        wl_t = const.tile([128, 128], BF16)
        nc.scalar.dma_start(wl_t[:], wl_d)
        issue_dma(1)
        issue_dma(2)

        # gap col layout: for group G (= 2gp+g), jj, q: col 4G+2jj+q;
        # partition 64a+m = patch (G, q, jj?, a)... (cols are summed
        # symmetrically on host, only ground<->sat pairing must match)
        gap = gpool.tile([128, NG * 4], F32)
        res = gpool.tile([128, 8], F32)
        wres = gpool.tile([128, 1], F32)  # warmup sink, never DMA'd

        # PE warm-up burst: keeps the PE busy from the end of the
        # framework prologue until the first conv1 matmul so the HAM
        # clock gate reaches 8/8 as early as possible
        junk = const.tile([128, 320], BF16, name="junk")
        nc.vector.memset(junk[:], 0.5)
        wps = pp1.tile([128, 320], F32, tag="ps1", name="warm_ps")
        for i in range(5):
            nc.tensor.matmul(
                wps[:],
                junk[:, 0:128],
                junk[:],
                start=(i == 0),
                stop=(i == 4),
            )
        wscr = spool.tile([128, 320], F32, tag="wscr")
        nc.scalar.activation(wscr[:], wps[:], SQUARE, accum_out=wres[:, 0:1])

        # greedy ACT/DVE load balancing on estimated busy-ns
        eng_ns = {"act": 0.0, "dve": 0.0}

        def evict_relu(dst, src):
            # conv1 eviction: relu, f32 PSUM -> bf16 SBUF
            if eng_ns["act"] + 630 <= eng_ns["dve"] + 660:
                eng_ns["act"] += 630
                nc.scalar.activation(dst, src, RELU)
            else:
                eng_ns["dve"] += 660
                nc.vector.tensor_scalar_max(dst, src, 0.0)

        def emit_conv1_g(gp, xt, h1, g):
            # conv1 for one group of a pair: 2 y-half waves of 4
            # concurrent 64x64 PE tiles (2R x 2cg); per-wave 2 PSUM
            # tiles so the 4-slot pool keeps 2 waves of slack, and the
            # g=0 block only needs half the pair's xi data (head ramp)
            for y0, nr in ((0, 16), (16, 15)):
                pss = {}
                for R in range(2):
                    ps = pp1.tile(
                        [128, nr, HOUT], F32, tag="ps1", name=f"c1_{g}{R}"
                    )
                    for cg in range(2):
                        nc.tensor.matmul(
                            ps[64 * cg : 64 * cg + 64, :, :],
                            w1_t[64 * R : 64 * R + KIM, :],
                            xt[
                                64 * R : 64 * R + KIM,
                                g,
                                cg,
                                y0 : y0 + nr,
                                :,
                            ],
                            start=True,
                            stop=True,
                            tile_position=(64 * R, 64 * cg),
                        )
                    pss[R] = ps
                for R in range(2):
                    evict_relu(
                        h1[:, g, R, y0 : y0 + nr, :], pss[R][:, :, :]
                    )

        c2_turn = [0]

        def emit_conv2_phase(gp, h1, g):
            # conv2 for one group of a pair: all 9 offsets, then evict.
            # Using only 2 of the 4 pp2 slots per phase leaves the pool
            # double-buffered across phases, so o=0 never stalls on
            # the previous phase's evictions.
            ps2s = {
                jj: pp2.tile(
                    [128, 2, COUT * COUT], F32, tag="ps2", name=f"ps2_{g}{jj}"
                )
                for jj in range(2)
            }
            for o in range(9):
                dy, dx = o // 3, o % 3
                for jj in range(2):
                    p0 = 64 * jj
                    nc.tensor.matmul(
                        ps2s[jj][:],
                        w2_t[p0 : p0 + 64, 128 * o : 128 * o + 128],
                        h1[
                            p0 : p0 + 64,
                            g,
                            :,
                            dy : dy + 29 : 2,
                            dx : dx + 29 : 2,
                        ],
                        start=(o == 0),
                        stop=(o == 8),
                        tile_position=(p0, 0),
                    )
            # eviction: relu(x + b2) then GAP sum into 2 gap columns.
            # NOTE: DVE accum_out is broken on TRN2 hardware (and
            # clobbers op1) -- only ACT may use accum_out. Plans:
            #  a) ACT relu->bf16 scratch + DVE segmented reduce
            #  b) DVE relu->bf16 scratch + DVE segmented reduce
            #  c) ACT in-place relu+bias+accum per q (no scratch)
            G = 2 * gp + g
            for jj in range(2):
                src = ps2s[jj]
                col = 4 * G + 2 * jj
                costs = {
                    "a": max(eng_ns["act"] + 630, eng_ns["dve"] + 613),
                    "b": eng_ns["dve"] + 1258,
                    "c": eng_ns["act"] + 1834,
                }
                plan = min(costs, key=costs.get)
                if plan == "c":
                    eng_ns["act"] += 1374
                    for q in range(2):
                        nc.scalar.activation(
                            src[:, q, :],
                            src[:, q, :],
                            RELU,
                            bias=b2_t[:],
                            accum_out=gap[:, col + q : col + q + 1],
                        )
                    continue
                scr = spool.tile(
                    [128, 2, COUT * COUT], BF16, tag="scr2", name="scr2"
                )
                if plan == "a":
                    eng_ns["act"] += 630
                    nc.scalar.activation(scr[:], src[:], RELU, bias=b2_t[:])
                else:
                    eng_ns["dve"] += 645
                    nc.vector.tensor_scalar(
                        scr[:], src[:], b2_t[:], 0.0, op0=ADD, op1=MAX
                    )
                eng_ns["dve"] += 613
                nc.vector.tensor_reduce(
                    gap[:, col : col + 2],
                    scr[:],
                    axis=mybir.AxisListType.X,
                    op=ADD,
                )

        def emit_chunk_diff(c):
            # feature diffs for 16 ground cols [16c, 16c+16) paired with
            # sat cols [64+16c, 80+16c), on GpSimd (SBUF-only engine)
            c0 = 16 * c
            dg = spool.tile([128, 16], F32, tag="dg", name=f"dg_{c}")
            dgb = spool.tile([128, 16], BF16, tag="dgb", name=f"dgb_{c}")
            nc.gpsimd.tensor_tensor(
                dg[:], gap[:, c0 : c0 + 16], gap[:, 64 + c0 : 80 + c0], op=SUB
            )
            nc.gpsimd.tensor_copy(dgb[:], dg[:])
            return dgb

        def emit_chunk_mm(c, dgb):
            # linear + square for a finished chunk (deferred so the PE
            # queue never waits on the GpSimd diff chain).
            for jj in range(2):
                p0 = 64 * jj
                ps3 = pp2.tile([128, 16], F32, tag="ps2", name=f"ps3_{c}{jj}")
                nc.tensor.matmul(
                    ps3[:],
                    wl_t[p0 : p0 + 64, :],
                    dgb[p0 : p0 + 64, :],
                    start=True,
                    stop=True,
                    tile_position=(p0, 0),
                )
                scr3 = spool.tile(
                    [128, 16], F32, tag="scr3", name=f"scr3_{c}{jj}"
                )
                nc.scalar.activation(
                    scr3[:], ps3[:], SQUARE, accum_out=res[:, 2 * c + jj : 2 * c + jj + 1]
                )
            # 2 unused pad allocations keep the 4-slot pp2 rotation
            # parity; they land on the in-flight g0 slots but are never
            # written or read, so they cannot stall anything
            for _pad in range(2):
                pp2.tile([128, 16], F32, tag="ps2", name=f"pad_{c}{_pad}")

        # software-pipelined emission: conv1 y-waves of pair i interleave
        # with the two conv2 group-phases of pair i-2; endgame chunk
        # after every 2nd sat group-pair
        pending = []
        done_sat = [0]
        chunk_q = []  # (c, dgb) whose PE part is deferred

        def flush_chunks():
            while chunk_q:
                c, dgb = chunk_q.pop(0)
                emit_chunk_mm(c, dgb)
                if c == 1:
                    nc.sync.dma_start(out_d[:, 0:4], res[:, 0:4])

        def after_conv2(gp2):
            if gp2 >= 8:
                done_sat[0] += 1
                if done_sat[0] % 2 == 0:
                    c = done_sat[0] // 2 - 1
                    chunk_q.append((c, emit_chunk_diff(c)))

        # per-iteration conv2 phase schedule: the pipeline fills with
        # two half-iterations (phase g0 of gp0 at i=1, g1 at i=2) so
        # the depth-2 restore bubble lands in the DMA-limited ramp
        phase_q = []  # (gp, h1, g) conv2 phases awaiting emission

        def pop_phase():
            if phase_q:
                gp2, h2, g2 = phase_q.pop(0)
                emit_conv2_phase(gp2, h2, g2)
                if g2 == 1:
                    after_conv2(gp2)

        for i in range(NGP):
            gp = GP_ORDER[i]
            xt = xi_tiles.pop(gp)
            h1 = hpool.tile(
                [128, 2, 2, HOUT, HOUT], BF16, tag="h1", name=f"h1_{gp}"
            )
            # pop phases BEFORE each conv1 block: a ready conv2 phase
            # must never sit behind a conv1 block still waiting on DMA
            if i >= 1:
                pop_phase()
            emit_conv1_g(gp, xt, h1, 0)
            if i + 3 < NGP:
                issue_dma(i + 3)
            if i >= 2:
                pop_phase()
            flush_chunks()
            emit_conv1_g(gp, xt, h1, 1)
            phase_q.append((gp, h1, 0))
            phase_q.append((gp, h1, 1))
        while phase_q:
            pop_phase()
            flush_chunks()
        nc.sync.dma_start(out_d[:, 4:8], res[:, 4:8])

    ndup = _dedupe_ldweights(nc)
    nsem = _strip_matmul_sem_incs(nc)
    print(
        f"[kernel] deduped {ndup} LDWEIGHTS, stripped {nsem} sem incs",
        file=sys.stderr,
    )
    nc.compile()
    return nc


def _prep_weights(w1, b1, w2, b2, wl):
    # conv1 im2col weights: [64R + 27a + (9c+3dy+3?dx), 32a+m]
    w1i = np.zeros((128, 64), np.float32)
    for a in range(2):
        for c in range(3):
            for dy in range(3):
                for dx in range(3):
                    w1i[27 * a + 9 * c + 3 * dy + dx, 32 * a : 32 * a + 32] = w1[
                        :, c, dy, dx
                    ]
        w1i[54, 32 * a : 32 * a + 32] = b1
    w1i[64:119] = w1i[0:55]

    w2blk = np.zeros((128, 9, 128), np.float32)
    for jj in range(2):
        for j in range(2):
            for c in range(32):
                for o in range(9):
                    dy, dx = o // 3, o % 3
                    w2blk[64 * jj + 32 * j + c, o, 64 * j : 64 * j + 64] = w2[
                        :, c, dy, dx
                    ]
    b2q = np.tile(b2, 2)[:, None].astype(np.float32)  # unscaled
    wlrep = np.zeros((128, 128), np.float32)
    wlrep[0:64] = wl.T
    wlrep[64:128] = wl.T
    return (
        w1i.astype(NPBF16),
        w2blk.reshape(128, 9 * 128).astype(NPBF16),
        np.ascontiguousarray(b2q),
        wlrep.astype(NPBF16),
    )


def _crop_all(images, kps):
    # images [B,3,H,W] f32; kps [NKP,2] normalized -> patches [NKP,B,3,P,P]
    hw = images.shape[-1]
    px = kps.astype(np.float32) * np.float32(hw)
    starts = np.clip(np.floor(px).astype(np.int32) - SIGMA, 0, hw - PATCH)
    out = np.empty((kps.shape[0], images.shape[0], 3, PATCH, PATCH), np.float32)
    for n in range(kps.shape[0]):
        x, y = int(starts[n, 0]), int(starts[n, 1])
        out[n] = images[:, :, y : y + PATCH, x : x + PATCH]
    return out


def _im2col_groups(pat):
    # pat [128,3,33,33] (one set for one core) -> [8, 2, 55, 2, 2, 31, 31]
    # (gp, R, im2col row (27a+9c+3dy+dx | 54=ones), g in-pair, cg, y, x)
    sw = np.lib.stride_tricks.sliding_window_view(pat, (HOUT, HOUT), axis=(2, 3))
    # sw[n, c, dy, dx, y, x] = pat[n, c, dy+y, dx+x]
    sw = sw.reshape(8, 2, 2, 2, 2, 27, HOUT, HOUT)  # (gp, g, R, cg, a, k, y, x)
    out = np.empty((8, 2, KIM, 2, 2, HOUT, HOUT), np.float32)
    # target row = 27a + k; dims (gp, R, a, k, g, cg, y, x)
    out[:, :, :54] = sw.transpose(0, 2, 4, 5, 1, 3, 6, 7).reshape(
        8, 2, 54, 2, 2, HOUT, HOUT
    )
    out[:, :, 54] = 1.0
    return out


def _make_in_maps(np_inputs):
    images_ground = np.asarray(np_inputs["images_ground"], np.float32)
    images_satellite = np.asarray(np_inputs["images_satellite"], np.float32)
    kg = np.asarray(np_inputs["keypoints_ground"], np.float32).reshape(-1, 2)
    ks = np.asarray(np_inputs["keypoints_satellite"], np.float32).reshape(-1, 2)
    w1 = np.asarray(np_inputs["w1"], np.float32)
    b1 = np.asarray(np_inputs["b1"], np.float32)
    w2 = np.asarray(np_inputs["w2"], np.float32)
    b2 = np.asarray(np_inputs["b2"], np.float32)
    wl = np.asarray(np_inputs["wl"], np.float32)

    pg = _crop_all(images_ground, kg)  # [256,4,3,33,33]
    ps = _crop_all(images_satellite, ks)
    w1i, w2blk, b2q, wlrep = _prep_weights(w1, b1, w2, b2, wl)

    in_maps = []
    for i in range(NCORES):
        sl = slice(i * KPC, (i + 1) * KPC)
        patg = pg[sl].reshape(NPATCH, 3, PATCH, PATCH)
        pats = ps[sl].reshape(NPATCH, 3, PATCH, PATCH)
        xi = np.concatenate(
            [_im2col_groups(patg), _im2col_groups(pats)], axis=0
        ).astype(NPBF16)
        in_maps.append(dict(xi=xi, w1=w1i, w2=w2blk, b2=b2q, wl=wlrep))
    return in_maps


def kernel(**inputs):
    in_maps = _make_in_maps(inputs)

    if "nc" not in _CACHE:
        _CACHE["nc"] = _build_graph()
    nc = _CACHE["nc"]

    results = bass_utils.run_bass_kernel_spmd(
        nc, in_maps, core_ids=list(range(NCORES))
    )
    total = np.float64(0.0)
    for r in results.results:
        total += np.asarray(r["out"], np.float64).sum()
    mse = total / (NKP * B * 128 * (COUT * COUT) ** 2)
    return np.asarray(mse, np.float32)


if __name__ == "__main__":
    rng = np.random.default_rng(0)
    ins = dict(
        images_ground=rng.standard_normal((B, 3, H, H)).astype(np.float32),
        images_satellite=rng.standard_normal((B, 3, H, H)).astype(np.float32),
        keypoints_ground=(0.2 + 0.6 * rng.random((B, K, 2))).astype(np.float32),
        keypoints_satellite=(0.2 + 0.6 * rng.random((B, K, 2))).astype(np.float32),
        w1=(rng.standard_normal((32, 3, 3, 3)) * 0.1).astype(np.float32),
        b1=np.zeros(32, np.float32),
        w2=(rng.standard_normal((64, 32, 3, 3)) * 0.05).astype(np.float32),
        b2=np.zeros(64, np.float32),
        wl=(rng.standard_normal((128, 64)) * 0.1).astype(np.float32),
        bl=np.zeros(128, np.float32),
        num_samples=K,
    )
    print("kernel out:", kernel(**ins))
